# revision 1
# baseline (speedup 1.0000x reference)
"""Causal attention block (LN -> QKV -> causal MHA -> out-proj) on 8 trn2
NeuronCores via Bass/Tile.

Sharding: core c handles batch b=c//2 and head-group g=c%2 (8 of 16 heads).
Data parallel over batch, tensor parallel over heads; the out-proj partial
sums (2 per batch) are reduced on the host during the gather, so the device
program needs no collectives and is pure SPMD.

Per-core layout choices:
  - x arrives host-transposed (d-major) so the QKV contraction has d on
    partitions with no on-device transpose.
  - LayerNorm is folded: gamma into W (host), beta and the mean correction
    enter each QKV accumulation as a K=2 seed matmul (rank-2 term), and the
    rstd scaling is a single elementwise multiply with a PE-broadcast row.
  - Scores are computed transposed (S^T = K^T.T @ Q^T per 128-ktok chunk)
    with 2 heads row-packed on the PE (K=64 each); exp runs on ACT straight
    out of PSUM with the 1/sqrt(dh) scale folded into the activation; the
    causal mask is a 0/1 multiply applied only to the 4 diagonal chunks per
    query tile (strictly-upper chunks are never computed).
  - The softmax denominator is obtained for free as a 65th ones-column of V
    in the P@V matmul; O^T is normalized with a reciprocal broadcast and fed
    (via a small DRAM scratch) as the stationary operand of the out-proj.
  - All matmuls use float32r (full-rate fp32 on trn2 for moving dim >= 256).
"""

import numpy as np

import concourse.bass as bass
import concourse.mybir as mybir
import concourse.tile as tile_mod

# ----------------------------------------------------------------------------
# Workaround for this walrus build rejecting instructions that carry more than
# MAX_WAITS semaphore waits ("Too many sync wait commands" in CoreV3GenImpl
# setupSyncWait — hit on Drain and Matmult/S3_LW encodings). Split excess
# waits onto single-wait NOP carrier instructions emitted just before the
# original instruction on the same engine: program order on the sequencer
# makes this semantically identical (waits are AND conditions).
# ----------------------------------------------------------------------------
_MAX_WAITS = 1
_orig_add_instruction = tile_mod.TileContext._add_instruction
_carrier_id = [0]


def _split_waits_add_instruction(self, inst):
    si = inst.sync_info
    if (
        si is not None
        and si.on_wait
        and len(si.on_wait) > _MAX_WAITS
        and inst.engine != mybir.EngineType.Unassigned
    ):
        waits = list(si.on_wait)
        keep = waits[-_MAX_WAITS:]
        for w in waits[:-_MAX_WAITS]:
            _carrier_id[0] += 1
            nop = mybir.InstNoOp(name=f"I-waitc-{_carrier_id[0]}")
            nop.engine = inst.engine
            nop.sync_info = mybir.SyncInfo(on_wait=[w], on_update=[])
            _orig_add_instruction(self, nop)
        inst.sync_info = mybir.SyncInfo(
            on_wait=keep,
            on_update=list(si.on_update) if si.on_update else [],
        )
    _orig_add_instruction(self, inst)


tile_mod.TileContext._add_instruction = _split_waits_add_instruction

from concourse.vector_clock import ScopedClock


def _patched_drain_and_barrier(self, tick_clock, wait_clock):
    # Same wait-splitting for the TileContext exit drain, which is emitted
    # after lowering (outside _add_instruction).
    nc = self.nc
    carrier = nc.sync.nop(nofuse=True)
    wait_clock.add_sem_waits(carrier.ins, ScopedClock({None: tick_clock.global_clock}))
    si = carrier.ins.sync_info
    waits = list(si.on_wait) if si is not None and si.on_wait else []
    if len(waits) > _MAX_WAITS:
        carrier.ins.sync_info = mybir.SyncInfo(
            on_wait=waits[:_MAX_WAITS],
            on_update=list(si.on_update) if si.on_update else [],
        )
        rest = waits[_MAX_WAITS:]
        while rest:
            extra = nc.sync.nop(nofuse=True)
            extra.ins.sync_info = mybir.SyncInfo(
                on_wait=rest[:_MAX_WAITS], on_update=[])
            rest = rest[_MAX_WAITS:]

    nc.sync.drain()
    nc.all_engine_barrier()
    assert self.sems is not None
    popped = nc._tile_sem_poison_stack.pop()
    assert popped is self._sem_poison
    nc.clear_and_free_semaphores(list(self.sems.allocated().values()))
    nc.all_engine_barrier()


tile_mod.TileContext._drain_and_barrier = _patched_drain_and_barrier

# ----------------------------------------------------------------------------

F32 = mybir.dt.float32
F32R = mybir.dt.float32r
BF16 = mybir.dt.bfloat16
ALU = mybir.AluOpType
ACT_EXP = mybir.ActivationFunctionType.Exp
ACT_SQRT = mybir.ActivationFunctionType.Sqrt
U32 = mybir.dt.uint32
ONE_BITS = int(np.float32(1.0).view(np.uint32))

B = 4
TOK = 2048
DIM = 1024
HEADS = 16
DH = 64
HEADS_PC = 8          # heads per core
INNER_PC = HEADS_PC * DH  # 512
NPAIRS = HEADS_PC // 2
QT = 512              # query tile (matmul moving dim)
KC = 128              # key-token chunk (contraction tile)
EPS = 1e-5
SCALE = DH ** -0.5


def _r(ap):
    """View an fp32 AP as float32r for full-rate PE matmuls."""
    return ap.bitcast(F32R)


def _bcast(ap, parts):
    """Partition-broadcast AP (stride-0 leading dim) for DMA reads of DRAM."""
    return bass.AP(tensor=ap.tensor, offset=ap.offset, ap=[[0, parts]] + list(ap.ap))


def build_program(tok=TOK):
    ntt = tok // KC      # 128-token tiles
    nqt = tok // QT      # query tiles
    nkd = DIM // 128     # d-contraction chunks

    nc = bass.Bass()
    xT = nc.declare_dram_parameter("xT", [DIM, tok], F32R, isOutput=False)
    w = nc.declare_dram_parameter("w", [DIM, 3 * INNER_PC], F32R, isOutput=False)
    seed = nc.declare_dram_parameter("seed", [2, 3 * INNER_PC], F32R, isOutput=False)
    wo = nc.declare_dram_parameter("wo", [INNER_PC, DIM], F32R, isOutput=False)
    masks = nc.declare_dram_parameter("masks", [4, KC, QT], BF16, isOutput=False)
    out = nc.declare_dram_parameter("out", [tok, DIM], F32, isOutput=True)
    oTs = nc.dram_tensor("oT_scratch", [NPAIRS, 128, tok], F32R)

    with tile_mod.TileContext(nc) as tc, nc.allow_low_precision(
            "f32r-tagged operand tiles; all matmul accumulation stays fp32 PSUM"):
        with (
            tc.tile_pool(name="const", bufs=1) as const,
            tc.tile_pool(name="vpool", bufs=ntt) as vpool,
            tc.tile_pool(name="qkT", bufs=2) as qkp,
        ):
            # ---------------- constants ----------------
            ones_row = const.tile([1, 128], F32R, tag="ones_row")
            nc.vector.memset(ones_row.bitcast(U32), ONE_BITS)
            ones_col = const.tile([128, 1], F32R, tag="ones_col")
            nc.vector.memset(ones_col.bitcast(U32), ONE_BITS)
            eps_t = const.tile([1, 1], F32, tag="eps")
            nc.vector.memset(eps_t, EPS)
            seed_sb = const.tile([2, 3 * INNER_PC], F32R, tag="seed")
            nc.sync.dma_start(out=seed_sb, in_=seed[:, :])
            mask_sb = const.tile([KC, 4, QT], BF16, tag="mask")
            for m in range(4):
                nc.sync.dma_start(out=mask_sb[:, m, :], in_=masks[m, :, :])
            onmr = const.tile([2, tok], F32R, tag="onmr")  # row0=1, row1=-mu*rstd
            nc.vector.memset(onmr[0:1, :].bitcast(U32), ONE_BITS)
            # stats rows (heavily reused; SBUF rows cost 8KB/partition each)
            r0 = const.tile([1, tok], F32R, tag="r0")  # sums -> mu
            r1 = const.tile([1, tok], F32R, tag="r1")  # sumsq -> ex2 -> var -> rstd
            r2 = const.tile([1, tok], F32R, tag="r2")  # musq -> std -> nmr

            with (
                tc.tile_pool(name="xt", bufs=nkd) as xtp,
                tc.tile_pool(name="wqk", bufs=nkd) as wqkp,
                tc.tile_pool(name="psb", bufs=3) as ppool,
                tc.tile_pool(name="osb", bufs=2) as osbp,
                tc.tile_pool(name="rb", bufs=2) as rbp,
            ):
                # ---------------- phase A: load x^T, LN stats ----------------
                xt = []
                for kc in range(nkd):
                    t = xtp.tile([128, tok], F32R, tag="xt")
                    nc.sync.dma_start(out=t, in_=xT[kc * 128:(kc + 1) * 128, :])
                    xt.append(t)

                with (
                    tc.tile_pool(name="ps_stats", bufs=1, space="PSUM") as pstat,
                    tc.tile_pool(name="sqp", bufs=2) as sqp,
                ):
                    sum_ps = pstat.tile([1, tok], F32, tag="sum")
                    sq_ps = pstat.tile([1, tok], F32, tag="sq")
                    for kc in range(nkd):
                        for nt in range(nqt):
                            sl = slice(nt * QT, (nt + 1) * QT)
                            sq_t = sqp.tile([128, QT], F32R, tag="sq_t")
                            nc.vector.tensor_mul(sq_t, xt[kc][:, sl], xt[kc][:, sl])
                            nc.tensor.matmul(
                                out=sum_ps[0:1, sl], lhsT=_r(ones_col),
                                rhs=_r(xt[kc][:, sl]),
                                start=(kc == 0), stop=(kc == nkd - 1))
                            nc.tensor.matmul(
                                out=sq_ps[0:1, sl], lhsT=_r(ones_col),
                                rhs=_r(sq_t),
                                start=(kc == 0), stop=(kc == nkd - 1))
                    nc.vector.tensor_copy(r0, sum_ps)
                    nc.vector.tensor_copy(r1, sq_ps)

                # stats postprocessing on [1, tok] rows:
                # r0: sums -> mu (-> stays mu)
                # r1: sumsq -> ex2 -> rstd
                # r2: musq -> var -> std -> nmr
                nc.vector.tensor_scalar_mul(r0, r0, 1.0 / DIM)      # mu
                nc.vector.tensor_scalar_mul(r1, r1, 1.0 / DIM)      # ex2
                nc.vector.tensor_mul(r2, r0, r0)                    # mu^2
                nc.vector.tensor_sub(r2, r1, r2)                    # var
                nc.scalar.activation(out=r2, in_=r2, func=ACT_SQRT,
                                     bias=eps_t, scale=1.0)         # std
                nc.vector.reciprocal(r1, r2)                        # rstd
                nc.vector.scalar_tensor_tensor(
                    out=r2, in0=r0, scalar=-1.0, in1=r1,
                    op0=ALU.mult, op1=ALU.mult)                     # -mu*rstd
                nc.sync.dma_start(out=onmr[1:2, :], in_=r2[0:1, :])

                # xn^T = x^T * rstd (broadcast along partitions via K=1 matmul)
                with tc.tile_pool(name="ps_bc", bufs=nqt, space="PSUM") as pbc:
                    for nt in range(nqt):
                        sl = slice(nt * QT, (nt + 1) * QT)
                        bc = pbc.tile([128, QT], F32, tag="bc")
                        nc.tensor.matmul(out=bc, lhsT=_r(ones_row),
                                         rhs=_r(r1[0:1, sl]),
                                         start=True, stop=True)
                        for kc in range(nkd):
                            nc.vector.tensor_mul(xt[kc][:, sl], xt[kc][:, sl], bc)

                # ---------------- phase B-V: V (token-major) + ones column ----
                v_sb = []
                with (
                    tc.tile_pool(name="wv", bufs=nkd) as wvp,
                    tc.tile_pool(name="ps_v", bufs=2, space="PSUM") as psv,
                ):
                    wv = []
                    for kc in range(nkd):
                        t = wvp.tile([128, INNER_PC], F32R, tag="wv")
                        nc.sync.dma_start(
                            out=t, in_=w[kc * 128:(kc + 1) * 128,
                                         2 * INNER_PC:3 * INNER_PC])
                        wv.append(t)
                    for tt in range(ntt):
                        tsl = slice(tt * KC, (tt + 1) * KC)
                        v_ps = psv.tile([128, INNER_PC], F32, tag="v_ps")
                        nc.tensor.matmul(
                            out=v_ps, lhsT=_r(onmr[:, tsl]),
                            rhs=_r(seed_sb[:, 2 * INNER_PC:3 * INNER_PC]),
                            start=True, stop=False)
                        for kc in range(nkd):
                            nc.tensor.matmul(
                                out=v_ps, lhsT=_r(xt[kc][:, tsl]), rhs=_r(wv[kc]),
                                start=False, stop=(kc == nkd - 1))
                        vt = vpool.tile([128, HEADS_PC * (DH + 1)], BF16, tag="v_sb")
                        v3 = vt.rearrange("p (h w) -> p h w", w=DH + 1)
                        nc.vector.tensor_copy(
                            v3[:, :, 0:DH],
                            v_ps.rearrange("p (h w) -> p h w", w=DH))
                        nc.vector.memset(v3[:, :, DH:DH + 1], 1.0)
                        v_sb.append(vt)

                # ---------------- phases B-QK + C, per head pair --------------
                with (
                    tc.tile_pool(name="ps_qk", bufs=1, space="PSUM") as psqk,
                    tc.tile_pool(name="ps_s", bufs=2, space="PSUM") as pss,
                    tc.tile_pool(name="ps_o", bufs=2, space="PSUM") as pso,
                    tc.tile_pool(name="ps_rb", bufs=1, space="PSUM") as psrb,
                ):
                    for p in range(NPAIRS):
                        # -- QK projection for this pair (128 cols each of Q, K)
                        wqk = []
                        for kc in range(nkd):
                            t = wqkp.tile([128, 256], F32R, tag="wqk")
                            dsl = slice(kc * 128, (kc + 1) * 128)
                            nc.sync.dma_start(
                                out=t[:, 0:128],
                                in_=w[dsl, p * 128:(p + 1) * 128])
                            nc.sync.dma_start(
                                out=t[:, 128:256],
                                in_=w[dsl, INNER_PC + p * 128:INNER_PC + (p + 1) * 128])
                            wqk.append(t)
                        qT = qkp.tile([128, tok], F32R, tag="qT")
                        kT = qkp.tile([128, tok], F32R, tag="kT")
                        for dst, cofs, sofs in (
                            (qT, 0, p * 128),
                            (kT, 128, INNER_PC + p * 128),
                        ):
                            for nt in range(nqt):
                                sl = slice(nt * QT, (nt + 1) * QT)
                                ps = psqk.tile([128, QT], F32, tag="qk_ps")
                                nc.tensor.matmul(
                                    out=ps, lhsT=_r(seed_sb[:, sofs:sofs + 128]),
                                    rhs=_r(onmr[:, sl]), start=True, stop=False)
                                for kc in range(nkd):
                                    nc.tensor.matmul(
                                        out=ps,
                                        lhsT=_r(wqk[kc][:, cofs:cofs + 128]),
                                        rhs=_r(xt[kc][:, sl]),
                                        start=False, stop=(kc == nkd - 1))
                                nc.vector.tensor_copy(dst[:, sl], ps)

                        # -- causal attention for heads (2p, 2p+1)
                        for t_i in range(nqt):
                            qsl = slice(t_i * QT, (t_i + 1) * QT)
                            nch = (t_i + 1) * QT // KC
                            o_ps = [pso.tile([DH + 1, QT], F32, tag="o_ps", name=f"o_ps{h}")
                                    for h in range(2)]
                            for c in range(nch):
                                csl = slice(c * KC, (c + 1) * KC)
                                s_ps = pss.tile([128, 2 * QT], F32, tag="s_ps")
                                nc.tensor.matmul(
                                    out=s_ps[:, 0:QT],
                                    lhsT=_r(kT[0:DH, csl]), rhs=_r(qT[0:DH, qsl]),
                                    start=True, stop=True)
                                nc.tensor.matmul(
                                    out=s_ps[:, QT:2 * QT],
                                    lhsT=_r(kT[DH:128, csl]), rhs=_r(qT[DH:128, qsl]),
                                    start=True, stop=True)
                                p_sb = ppool.tile([128, 2 * QT], BF16, tag="p_sb")
                                nc.scalar.activation(out=p_sb, in_=s_ps,
                                                     func=ACT_EXP, scale=SCALE)
                                m = c - (nch - 4)
                                if m >= 0:
                                    nc.vector.tensor_mul(
                                        p_sb[:, 0:QT], p_sb[:, 0:QT], mask_sb[:, m, :])
                                    nc.vector.tensor_mul(
                                        p_sb[:, QT:2 * QT], p_sb[:, QT:2 * QT],
                                        mask_sb[:, m, :])
                                for h in range(2):
                                    hc = (2 * p + h) * (DH + 1)
                                    nc.tensor.matmul(
                                        out=o_ps[h],
                                        lhsT=v_sb[c][:, hc:hc + DH + 1],
                                        rhs=p_sb[:, h * QT:(h + 1) * QT],
                                        start=(c == 0), stop=(c == nch - 1))
                            for h in range(2):
                                recip = rbp.tile([1, QT], F32R, tag="recip")
                                nc.vector.reciprocal(recip, o_ps[h][DH:DH + 1, :])
                                rb_ps = psrb.tile([DH, QT], F32, tag="rb_ps")
                                nc.tensor.matmul(
                                    out=rb_ps, lhsT=ones_row[:, 0:DH], rhs=recip,
                                    start=True, stop=True)
                                rb = rbp.tile([DH, QT], F32, tag="rb")
                                nc.vector.tensor_copy(rb, rb_ps)
                                osb = osbp.tile([DH, QT], F32R, tag="osb")
                                nc.vector.scalar_tensor_tensor(
                                    out=osb, in0=o_ps[h][0:DH, :], scalar=1.0,
                                    in1=rb, op0=ALU.mult, op1=ALU.mult)
                                nc.sync.dma_start(
                                    out=oTs[p, h * DH:(h + 1) * DH, qsl], in_=osb)

            # ---------------- phase D: out projection ----------------
            with (
                tc.tile_pool(name="wo_sb", bufs=NPAIRS) as wop,
                tc.tile_pool(name="oL", bufs=2 * NPAIRS) as olp,
                tc.tile_pool(name="out_sb", bufs=3) as outp,
                tc.tile_pool(name="ps_out", bufs=2, space="PSUM") as psout,
            ):
                wos = []
                for p_i in range(NPAIRS):
                    t = wop.tile([128, DIM], F32R, tag="wo_sb")
                    nc.sync.dma_start(out=t, in_=wo[p_i * 128:(p_i + 1) * 128, :])
                    wos.append(t)
                for tt in range(ntt):
                    tsl = slice(tt * KC, (tt + 1) * KC)
                    ol = []
                    for p_i in range(NPAIRS):
                        t = olp.tile([128, KC], F32R, tag="oL")
                        nc.sync.dma_start(out=t, in_=oTs[p_i, :, tsl])
                        ol.append(t)
                    for nb in range(DIM // QT):
                        nsl = slice(nb * QT, (nb + 1) * QT)
                        ps = psout.tile([128, QT], F32, tag="out_ps")
                        for p_i in range(NPAIRS):
                            nc.tensor.matmul(
                                out=ps, lhsT=_r(ol[p_i]), rhs=_r(wos[p_i][:, nsl]),
                                start=(p_i == 0), stop=(p_i == NPAIRS - 1))
                        ob = outp.tile([128, QT], F32, tag="out_sb")
                        nc.vector.tensor_copy(ob, ps)
                        nc.sync.dma_start(out=out[tsl, nsl], in_=ob)

    return nc


def make_masks():
    import ml_dtypes

    j = np.arange(KC)[:, None]
    i = np.arange(QT)[None, :]
    return np.stack(
        [(i >= j + 128 * m) for m in range(4)]).astype(ml_dtypes.bfloat16)


def make_in_maps(x, ln_gamma, ln_beta, w_qkv, w_out):
    x = np.asarray(x, np.float32)
    g_ = np.asarray(ln_gamma, np.float32)
    b_ = np.asarray(ln_beta, np.float32)
    w_qkv = np.asarray(w_qkv, np.float32)
    w_out = np.asarray(w_out, np.float32)
    masks = make_masks()
    in_maps = []
    for c in range(8):
        b = c // 2
        g = c % 2
        cs = slice(g * INNER_PC, (g + 1) * INNER_PC)
        Wraw = np.concatenate(
            [w_qkv[:, 0 * DIM:1 * DIM][:, cs],
             w_qkv[:, 1 * DIM:2 * DIM][:, cs],
             w_qkv[:, 2 * DIM:3 * DIM][:, cs]], axis=1)
        Wp = (Wraw * g_[:, None]).astype(np.float32)
        seed = np.stack([b_ @ Wraw, Wp.sum(axis=0)]).astype(np.float32)
        in_maps.append({
            "xT": np.ascontiguousarray(x[b].T),
            "w": np.ascontiguousarray(Wp),
            "seed": seed,
            "wo": np.ascontiguousarray(w_out[cs, :]),
            "masks": masks,
        })
    return in_maps


_PROG = None


def kernel(x, ln_gamma, ln_beta, w_qkv, w_out):
    global _PROG
    from concourse.bass_utils import run_bass_kernel_spmd

    if _PROG is None:
        _PROG = build_program(TOK)
    in_maps = make_in_maps(x, ln_gamma, ln_beta, w_qkv, w_out)
    res = run_bass_kernel_spmd(_PROG, in_maps, list(range(8)))
    parts = [res.results[c]["out"] for c in range(8)]
    out = np.empty((B, TOK, DIM), np.float32)
    for b in range(B):
        out[b] = parts[2 * b] + parts[2 * b + 1]
    return out



# revision 19
# speedup vs baseline: 1.1650x; 1.1650x over previous
"""Causal attention block (LN -> QKV -> causal MHA -> out-proj) on 8 trn2
NeuronCores via Bass/Tile.

Sharding: core c handles batch b=c//2 and head-group g=c%2 (8 of 16 heads).
Data parallel over batch, tensor parallel over heads; the out-proj partial
sums (2 per batch) are reduced on the host during the gather, so the device
program needs no collectives and is pure SPMD.

Per-core pipeline (all matmul operands bf16, fp32 PSUM accumulation):
  A) x^T arrives host-transposed d-major in bf16. LN stats via ones-column
     matmuls into [1, 2048] PSUM rows; the rows are PE-broadcast (with the
     1/D mean division folded into the broadcast operand) to [128, 512]
     tiles and the whole mu/var/rstd postprocess runs on those wide tiles
     instead of slow single-partition rows. xn = x*rstd - mu*rstd is then
     materialized in place (DVE mult + Pool sub), which removes every seed
     matmul from the projections.
  B) V (token-major, with a 65th ones column for the softmax denominator)
     and Q^T/K^T for all 4 head pairs. QK PSUM->SBUF copies run on the ACT
     engine as Identity-with-bias, which also applies the LN beta term
     through the projection for free; V copies run on Pool (beta folded as
     a broadcast add).
  C) Attention per query tile x head pair: S^T per 128-ktok chunk via two
     K=64 matmuls (2 row-packed heads), exp on ACT straight out of PSUM
     with the 1/sqrt(dh) scale folded in, 0/1 mask multiplies (DVE, bf16)
     on the 4 diagonal chunks only, then P^T@V accumulation with the ones
     column producing the denominator. Normalization uses one K=2 select
     matmul to broadcast both heads' reciprocals at once. O^T stays in
     SBUF (no DRAM scratch).
  D) The out projection is interleaved into the attention chunk stream of
     the next query tile (attention is ACT-paced, so these matmuls fill
     PE slack); PSUM->SBUF copies on Pool, then DMA to DRAM in fp32.
"""

import numpy as np

import concourse.bass as bass
import concourse.mybir as mybir
import concourse.tile as tile_mod

# ----------------------------------------------------------------------------
# Workaround for this walrus build rejecting instructions that carry more than
# MAX_WAITS semaphore waits ("Too many sync wait commands" in CoreV3GenImpl
# setupSyncWait — hit on Drain and Matmult/S3_LW encodings). Split excess
# waits onto single-wait NOP carrier instructions emitted just before the
# original instruction on the same engine: program order on the sequencer
# makes this semantically identical (waits are AND conditions).
# ----------------------------------------------------------------------------
_MAX_WAITS = 1
_orig_add_instruction = tile_mod.TileContext._add_instruction
_carrier_id = [0]


def _split_waits_add_instruction(self, inst):
    si = inst.sync_info
    if (
        si is not None
        and si.on_wait
        and len(si.on_wait) > _MAX_WAITS
        and inst.engine != mybir.EngineType.Unassigned
    ):
        waits = list(si.on_wait)
        keep = waits[-_MAX_WAITS:]
        for w in waits[:-_MAX_WAITS]:
            _carrier_id[0] += 1
            nop = mybir.InstNoOp(name=f"I-waitc-{_carrier_id[0]}")
            nop.engine = inst.engine
            nop.sync_info = mybir.SyncInfo(on_wait=[w], on_update=[])
            _orig_add_instruction(self, nop)
        inst.sync_info = mybir.SyncInfo(
            on_wait=keep,
            on_update=list(si.on_update) if si.on_update else [],
        )
    _orig_add_instruction(self, inst)


tile_mod.TileContext._add_instruction = _split_waits_add_instruction

from concourse.vector_clock import ScopedClock


def _patched_drain_and_barrier(self, tick_clock, wait_clock):
    # Same wait-splitting for the TileContext exit drain, which is emitted
    # after lowering (outside _add_instruction).
    nc = self.nc
    carrier = nc.sync.nop(nofuse=True)
    wait_clock.add_sem_waits(carrier.ins, ScopedClock({None: tick_clock.global_clock}))
    si = carrier.ins.sync_info
    waits = list(si.on_wait) if si is not None and si.on_wait else []
    if len(waits) > _MAX_WAITS:
        carrier.ins.sync_info = mybir.SyncInfo(
            on_wait=waits[:_MAX_WAITS],
            on_update=list(si.on_update) if si.on_update else [],
        )
        rest = waits[_MAX_WAITS:]
        while rest:
            extra = nc.sync.nop(nofuse=True)
            extra.ins.sync_info = mybir.SyncInfo(
                on_wait=rest[:_MAX_WAITS], on_update=[])
            rest = rest[_MAX_WAITS:]

    nc.sync.drain()
    nc.all_engine_barrier()
    assert self.sems is not None
    popped = nc._tile_sem_poison_stack.pop()
    assert popped is self._sem_poison
    nc.clear_and_free_semaphores(list(self.sems.allocated().values()))
    nc.all_engine_barrier()


tile_mod.TileContext._drain_and_barrier = _patched_drain_and_barrier

# ----------------------------------------------------------------------------

F32 = mybir.dt.float32
F32R = mybir.dt.float32r
BF16 = mybir.dt.bfloat16
ALU = mybir.AluOpType
ACT_EXP = mybir.ActivationFunctionType.Exp
ACT_SQRT = mybir.ActivationFunctionType.Sqrt
ACT_IDENT = mybir.ActivationFunctionType.Identity
U32 = mybir.dt.uint32
ONE_BITS = int(np.float32(1.0).view(np.uint32))

B = 4
TOK = 2048
DIM = 1024
HEADS = 16
DH = 64
HEADS_PC = 8          # heads per core
INNER_PC = HEADS_PC * DH  # 512
NPAIRS = HEADS_PC // 2
QT = 512              # query tile (matmul moving dim)
KC = 128              # key-token chunk (contraction tile)
NKD = DIM // 128      # d-contraction chunks
NQT = TOK // QT       # query tiles
NTT = TOK // KC       # 128-token tiles
EPS = 1e-5
SCALE = DH ** -0.5


def _r(ap):
    """View an fp32 AP as float32r for full-rate PE matmuls."""
    return ap.bitcast(F32R)


def build_program():
    nc = bass.Bass()
    xT = nc.declare_dram_parameter("xT", [DIM, TOK], BF16, isOutput=False)
    w = nc.declare_dram_parameter("w", [DIM, 3 * INNER_PC], BF16, isOutput=False)
    wo = nc.declare_dram_parameter("wo", [INNER_PC, DIM], BF16, isOutput=False)
    masks = nc.declare_dram_parameter("masks", [4, KC, QT], BF16, isOutput=False)
    betaqk = nc.declare_dram_parameter("betaqk", [128, 2 * NPAIRS], F32,
                                       isOutput=False)
    betav = nc.declare_dram_parameter("betav", [1, INNER_PC], F32R,
                                      isOutput=False)
    selm = nc.declare_dram_parameter("selm", [2, 128], F32R, isOutput=False)
    out = nc.declare_dram_parameter("out", [TOK, DIM], F32, isOutput=True)

    with tile_mod.TileContext(nc) as tc, nc.allow_low_precision(
            "bf16 operand tiles; all matmul accumulation stays fp32 PSUM"):
        with (
            tc.tile_pool(name="const", bufs=1) as const,
            tc.tile_pool(name="xt", bufs=NKD) as xtp,
            tc.tile_pool(name="wsb", bufs=NKD) as wsp,
            tc.tile_pool(name="wo_sb", bufs=NPAIRS) as wop,
            tc.tile_pool(name="vpool", bufs=NTT) as vpool,
            tc.tile_pool(name="qkT", bufs=NPAIRS) as qkp,
            tc.tile_pool(name="oT", bufs=NPAIRS) as otp,
            tc.tile_pool(name="bc", bufs=NQT) as bcp,
        ):
            # ---------------- constants + full prefetch ----------------
            ones_col = const.tile([128, 1], BF16, tag="ones_col")
            nc.vector.memset(ones_col, 1.0)
            ones_row = const.tile([1, 128], F32R, tag="ones_row")
            nc.vector.memset(ones_row.bitcast(U32), ONE_BITS)
            eps128 = const.tile([128, 1], F32, tag="eps")
            nc.vector.memset(eps128, EPS)
            bqk_sb = const.tile([128, 2 * NPAIRS], F32, tag="bqk")
            nc.sync.dma_start(out=bqk_sb, in_=betaqk[:, :])
            # head-select matrix for the two-head reciprocal broadcast
            sel_sb = const.tile([2, 128], F32R, tag="sel")
            nc.sync.dma_start(out=sel_sb, in_=selm[:, :])
            bv_row = const.tile([1, INNER_PC], F32R, tag="bv_row")
            nc.sync.dma_start(out=bv_row, in_=betav[:, :])
            mask_sb = const.tile([KC, 4, QT], BF16, tag="mask")
            for m in range(4):
                nc.sync.dma_start(out=mask_sb[:, m, :], in_=masks[m, :, :])

            xt = []
            for kc in range(NKD):
                t = xtp.tile([128, TOK], BF16, tag="xt")
                nc.sync.dma_start(out=t, in_=xT[kc * 128:(kc + 1) * 128, :])
                xt.append(t)
            wsb = []
            for kc in range(NKD):
                t = wsp.tile([128, 3 * INNER_PC], BF16, tag="wsb")
                nc.sync.dma_start(out=t, in_=w[kc * 128:(kc + 1) * 128, :])
                wsb.append(t)
            wos = []
            for p in range(NPAIRS):
                t = wop.tile([128, DIM], BF16, tag="wo_sb")
                nc.sync.dma_start(out=t, in_=wo[p * 128:(p + 1) * 128, :])
                wos.append(t)

            # persistent per-pair tiles
            qT = [qkp.tile([128, TOK], BF16, tag="qT", name=f"qT{p}")
                  for p in range(NPAIRS)]
            kT = [qkp.tile([128, TOK], BF16, tag="kT", name=f"kT{p}")
                  for p in range(NPAIRS)]
            oT = [otp.tile([128, TOK], BF16, tag="oT", name=f"oT{p}")
                  for p in range(NPAIRS)]
            bv_bc = const.tile([128, INNER_PC], F32, tag="bv_bc")

            # ---------------- phase A: LN stats + xn ----------------
            sum_r = const.tile([1, TOK], BF16, tag="sum_r")
            sq_r = const.tile([1, TOK], BF16, tag="sq_r")
            with (
                tc.tile_pool(name="sqp", bufs=3) as sqp,
            ):
                with tc.tile_pool(name="ps_stats", bufs=1,
                                  space="PSUM") as pstat:
                    sum_ps = pstat.tile([1, TOK], F32, tag="sum")
                    sq_ps = pstat.tile([1, TOK], F32, tag="sq")
                    for kc in range(NKD):
                        for nt in range(NQT):
                            sl = slice(nt * QT, (nt + 1) * QT)
                            sq_t = sqp.tile([128, QT], BF16, tag="sq_t")
                            nc.scalar.square(sq_t, xt[kc][:, sl])
                            nc.tensor.matmul(
                                out=sum_ps[0:1, sl], lhsT=ones_col,
                                rhs=xt[kc][:, sl],
                                start=(kc == 0), stop=(kc == NKD - 1))
                            nc.tensor.matmul(
                                out=sq_ps[0:1, sl], lhsT=ones_col,
                                rhs=sq_t,
                                start=(kc == 0), stop=(kc == NKD - 1))
                    for nt in range(NQT):
                        sl = slice(nt * QT, (nt + 1) * QT)
                        nc.vector.tensor_copy(sum_r[0:1, sl], sum_ps[0:1, sl])
                        nc.scalar.copy(sq_r[0:1, sl], sq_ps[0:1, sl])

                # broadcast sum/sumsq rows (with the 1/D fold) to [128, QT]
                # and run the whole stats chain on wide tiles
                bc_rstd, bc_nmr = [], []
                with (
                    tc.tile_pool(name="ps_bc", bufs=4, space="PSUM") as pbc,
                    tc.tile_pool(name="stw", bufs=2) as stw,
                ):
                    inv_bf = const.tile([1, 128], BF16, tag="inv_bf")
                    nc.vector.memset(inv_bf, 1.0 / DIM)
                    for nt in range(NQT):
                        sl = slice(nt * QT, (nt + 1) * QT)
                        mu_ps = pbc.tile([128, QT], F32, tag="bc_ps")
                        nc.tensor.matmul(out=mu_ps, lhsT=inv_bf,
                                         rhs=sum_r[0:1, sl],
                                         start=True, stop=True)
                        ex_ps = pbc.tile([128, QT], F32, tag="bc_ps")
                        nc.tensor.matmul(out=ex_ps, lhsT=inv_bf,
                                         rhs=sq_r[0:1, sl],
                                         start=True, stop=True)
                        var = stw.tile([128, QT], F32, tag="var")
                        nc.scalar.square(var, mu_ps)
                        nc.vector.tensor_sub(var, ex_ps, var)
                        nc.scalar.activation(out=var, in_=var, func=ACT_SQRT,
                                             bias=eps128, scale=1.0)  # std
                        rstd = stw.tile([128, QT], F32, tag="rstd")
                        nc.vector.reciprocal(rstd, var)
                        br = bcp.tile([128, QT], BF16, tag="bc_rstd")
                        nc.vector.tensor_copy(br, rstd)
                        bc_rstd.append(br)
                        bn = bcp.tile([128, QT], BF16, tag="bc_nmr")
                        nc.vector.tensor_mul(bn, mu_ps, rstd)  # mu*rstd
                        bc_nmr.append(bn)
                    bv_ps = pbc.tile([128, INNER_PC], F32, tag="bv_ps",
                                     bufs=1)
                    nc.tensor.matmul(out=bv_ps, lhsT=ones_row,
                                     rhs=bv_row, start=True, stop=True)
                    nc.vector.tensor_copy(bv_bc, bv_ps)

                # xn = x*rstd - mu*rstd, in place (mult on DVE, sub on Pool)
                for kc in range(NKD):
                    for nt in range(NQT):
                        sl = slice(nt * QT, (nt + 1) * QT)
                        nc.vector.tensor_mul(xt[kc][:, sl], xt[kc][:, sl],
                                             bc_rstd[nt])
                        nc.vector.tensor_sub(xt[kc][:, sl], xt[kc][:, sl],
                                             bc_nmr[nt])

            # ---------------- phase B: V then Q^T/K^T projections ----------
            v_sb = []
            with (
                tc.tile_pool(name="ps_v", bufs=2, space="PSUM") as psv,
                tc.tile_pool(name="ps_qk", bufs=3, space="PSUM") as psqk,
            ):
                for tt in range(NTT):
                    tsl = slice(tt * KC, (tt + 1) * KC)
                    v_ps = psv.tile([128, INNER_PC], F32, tag="v_ps")
                    for kc in range(NKD):
                        nc.tensor.matmul(
                            out=v_ps, lhsT=xt[kc][:, tsl],
                            rhs=wsb[kc][:, 2 * INNER_PC:3 * INNER_PC],
                            start=(kc == 0), stop=(kc == NKD - 1))
                    vt = vpool.tile([128, HEADS_PC * (DH + 1)], BF16, tag="v_sb")
                    v3 = vt.rearrange("p (h w) -> p h w", w=DH + 1)
                    nc.vector.tensor_add(
                        v3[:, :, 0:DH],
                        v_ps.rearrange("p (h w) -> p h w", w=DH),
                        bv_bc.rearrange("p (h w) -> p h w", w=DH))
                    nc.vector.memset(v3[:, :, DH:DH + 1], 1.0)
                    v_sb.append(vt)

                for p in range(NPAIRS):
                    for di, (dst, cofs) in enumerate(
                        ((qT[p], p * 128), (kT[p], INNER_PC + p * 128)),
                    ):
                        for nt in range(NQT):
                            sl = slice(nt * QT, (nt + 1) * QT)
                            ps = psqk.tile([128, QT], F32, tag="qk_ps")
                            for kc in range(NKD):
                                nc.tensor.matmul(
                                    out=ps,
                                    lhsT=wsb[kc][:, cofs:cofs + 128],
                                    rhs=xt[kc][:, sl],
                                    start=(kc == 0), stop=(kc == NKD - 1))
                            nc.scalar.activation(
                                out=dst[:, sl], in_=ps, func=ACT_IDENT,
                                bias=bqk_sb[:, 2 * p + di:2 * p + di + 1],
                                scale=1.0)

            # -------- phases C+D: attention with interleaved out-proj -------
            # D work unit: one (token-128-tile, dim-half) PSUM accumulation
            # over the 4 pair chunks of O^T, copied to SBUF on Pool, DMA'd.
            with (
                tc.tile_pool(name="ps_s", bufs=2, space="PSUM") as pss,
                tc.tile_pool(name="ps_o", bufs=3, space="PSUM") as pso,
                tc.tile_pool(name="ps_misc", bufs=1, space="PSUM") as pmisc,
                tc.tile_pool(name="psb", bufs=4) as ppool,
                tc.tile_pool(name="rbp", bufs=2) as rbp,
                tc.tile_pool(name="rec", bufs=2) as recp,
                tc.tile_pool(name="out_sb", bufs=3) as outp,
            ):
                def d_unit(t_i, u):
                    # out tokens [t_i*QT + (u//2)*KC ...), dim half u%2
                    tt = t_i * NQT + u // 2
                    nb = u % 2
                    tsl = slice(tt * KC, (tt + 1) * KC)
                    nsl = slice(nb * QT, (nb + 1) * QT)
                    ps = pmisc.tile([128, QT], F32, tag="d_ps")
                    for p_i in range(NPAIRS):
                        nc.tensor.matmul(
                            out=ps, lhsT=oT[p_i][:, tsl],
                            rhs=wos[p_i][:, nsl],
                            start=(p_i == 0), stop=(p_i == NPAIRS - 1))
                    ob = outp.tile([128, QT], F32, tag="out_sb")
                    nc.vector.tensor_copy(ob, ps)
                    nc.sync.dma_start(out=out[tsl, nsl], in_=ob)

                for t_i in range(NQT):
                    qsl = slice(t_i * QT, (t_i + 1) * QT)
                    nch = (t_i + 1) * NQT
                    # spread the 8 D units of the previous query tile across
                    # this tile's chunk stream (attention is ACT-paced)
                    d_total = 2 * NQT if t_i > 0 else 0
                    d_done = 0
                    chunks_all = NPAIRS * nch
                    ci = 0
                    for p in range(NPAIRS):
                        o_ps = [pso.tile([DH + 1, QT], F32, tag="o_ps",
                                         name=f"o_ps{h}") for h in range(2)]
                        for c in range(nch):
                            csl = slice(c * KC, (c + 1) * KC)
                            s_ps = pss.tile([128, 2 * QT], F32, tag="s_ps")
                            nc.tensor.matmul(
                                out=s_ps[:, 0:QT],
                                lhsT=kT[p][0:DH, csl], rhs=qT[p][0:DH, qsl],
                                start=True, stop=True)
                            nc.tensor.matmul(
                                out=s_ps[:, QT:2 * QT],
                                lhsT=kT[p][DH:128, csl], rhs=qT[p][DH:128, qsl],
                                start=True, stop=True)
                            p_sb = ppool.tile([128, 2 * QT], BF16, tag="p_sb")
                            nc.scalar.activation(out=p_sb, in_=s_ps,
                                                 func=ACT_EXP, scale=SCALE)
                            m = c - (nch - 4)
                            if m >= 0:
                                nc.vector.tensor_mul(
                                    p_sb[:, 0:QT], p_sb[:, 0:QT],
                                    mask_sb[:, m, :])
                                nc.vector.tensor_mul(
                                    p_sb[:, QT:2 * QT], p_sb[:, QT:2 * QT],
                                    mask_sb[:, m, :])
                            for h in range(2):
                                hc = (2 * p + h) * (DH + 1)
                                nc.tensor.matmul(
                                    out=o_ps[h],
                                    lhsT=v_sb[c][:, hc:hc + DH + 1],
                                    rhs=p_sb[:, h * QT:(h + 1) * QT],
                                    start=(c == 0), stop=(c == nch - 1))
                            ci += 1
                            # interleave prior tile's out-proj
                            want = d_total * ci // chunks_all
                            while d_done < want:
                                d_unit(t_i - 1, d_done)
                                d_done += 1
                        # normalize both heads of this pair: reciprocals of
                        # the two denominator rows, one K=2 select matmul
                        # broadcasts both to [128, QT]
                        rec2 = recp.tile([2, QT], F32R, tag="rec2")
                        nc.vector.reciprocal(rec2[0:1, :],
                                             o_ps[0][DH:DH + 1, :])
                        rec_b = recp.tile([1, QT], F32R, tag="rec_b")
                        nc.vector.reciprocal(rec_b,
                                             o_ps[1][DH:DH + 1, :])
                        nc.sync.dma_start(out=rec2[1:2, :], in_=rec_b)
                        rb_ps = pso.tile([128, QT], F32, tag="o_ps",
                                         name="rb_ps")
                        nc.tensor.matmul(out=rb_ps, lhsT=sel_sb, rhs=rec2,
                                         start=True, stop=True)
                        rb = rbp.tile([128, QT], F32, tag="rb")
                        nc.vector.tensor_copy(rb, rb_ps)
                        for h in range(2):
                            nc.vector.scalar_tensor_tensor(
                                out=oT[p][h * DH:(h + 1) * DH, qsl],
                                in0=o_ps[h][0:DH, :], scalar=1.0,
                                in1=rb[h * DH:(h + 1) * DH, :],
                                op0=ALU.mult, op1=ALU.mult)
                    while d_done < d_total:
                        d_unit(t_i - 1, d_done)
                        d_done += 1
                # trailing out-proj for the last query tile
                for u in range(2 * NQT):
                    d_unit(NQT - 1, u)

    return nc


def make_masks():
    import ml_dtypes

    j = np.arange(KC)[:, None]
    i = np.arange(QT)[None, :]
    return np.stack(
        [(i >= j + 128 * m) for m in range(4)]).astype(ml_dtypes.bfloat16)


def make_in_maps(x, ln_gamma, ln_beta, w_qkv, w_out):
    import ml_dtypes

    bf16 = ml_dtypes.bfloat16
    x = np.asarray(x, np.float32)
    g_ = np.asarray(ln_gamma, np.float32)
    b_ = np.asarray(ln_beta, np.float32)
    w_qkv = np.asarray(w_qkv, np.float32)
    w_out = np.asarray(w_out, np.float32)
    masks = make_masks()
    in_maps = []
    for c in range(8):
        b = c // 2
        g = c % 2
        cs = slice(g * INNER_PC, (g + 1) * INNER_PC)
        Wraw = np.concatenate(
            [w_qkv[:, 0 * DIM:1 * DIM][:, cs],
             w_qkv[:, 1 * DIM:2 * DIM][:, cs],
             w_qkv[:, 2 * DIM:3 * DIM][:, cs]], axis=1)
        Wp = Wraw * g_[:, None]
        bqkv = b_ @ Wraw  # [3*INNER_PC]
        betaqk = np.empty((128, 2 * NPAIRS), np.float32)
        for p in range(NPAIRS):
            betaqk[:, 2 * p] = bqkv[p * 128:(p + 1) * 128]
            betaqk[:, 2 * p + 1] = bqkv[INNER_PC + p * 128:
                                        INNER_PC + (p + 1) * 128]
        betav = np.ascontiguousarray(
            bqkv[2 * INNER_PC:3 * INNER_PC][None, :]).astype(np.float32)
        selm = np.zeros((2, 128), np.float32)
        selm[0, 0:64] = 1.0
        selm[1, 64:128] = 1.0
        in_maps.append({
            "xT": np.ascontiguousarray(x[b].T).astype(bf16),
            "w": Wp.astype(bf16),
            "wo": np.ascontiguousarray(w_out[cs, :]).astype(bf16),
            "masks": masks,
            "betaqk": betaqk,
            "betav": betav,
            "selm": selm,
        })
    return in_maps


_PROG = None


def kernel(x, ln_gamma, ln_beta, w_qkv, w_out):
    global _PROG
    from concourse.bass_utils import run_bass_kernel_spmd

    if _PROG is None:
        _PROG = build_program()
    in_maps = make_in_maps(x, ln_gamma, ln_beta, w_qkv, w_out)
    res = run_bass_kernel_spmd(_PROG, in_maps, list(range(8)))
    parts = [res.results[c]["out"] for c in range(8)]
    out = np.empty((B, TOK, DIM), np.float32)
    for b in range(B):
        out[b] = parts[2 * b] + parts[2 * b + 1]
    return out


# revision 27
# speedup vs baseline: 1.7430x; 1.4961x over previous
"""Causal attention block (LN -> QKV -> causal MHA -> out-proj) on 8 trn2
NeuronCores via Bass/Tile.

Sharding: core c handles batch b=c//2 and head-group g=c%2 (8 of 16 heads).
Data parallel over batch, tensor parallel over heads; the out-proj partial
sums (2 per batch) are reduced on the host during the gather, so the device
program needs no collectives and is pure SPMD.

Per-core pipeline (all matmul operands bf16, fp32 PSUM accumulation):
  A) x^T arrives host-transposed d-major in bf16. LN stats via ones-column
     matmuls into [1, 2048] PSUM rows; the rows are PE-broadcast (with the
     1/D mean division folded into the broadcast operand) to [128, 512]
     tiles and the whole mu/var/rstd postprocess runs on those wide tiles.
     xn = x*rstd - mu*rstd is materialized in place (2 DVE ops per tile,
     earliest token slice first so phase B can start), which removes every
     seed matmul from the projections.
  B) V (token-major, with a 65th ones column for the softmax denominator)
     and Q^T/K^T for all 4 head pairs. QK PSUM->SBUF copies run on the ACT
     engine as Identity-with-bias (applies the LN beta term for free).
  C) Attention per query tile x head pair, software-pipelined so the PE
     never waits on the ACT exp: scores of chunk c+1 are emitted before
     P@V of chunk c. Diagonal chunks compute/exp/mask only the causally
     live [128m:512] sub-rectangle. The softmax denominator rides as a
     65th ones-column of V; normalization is decoupled from the PSUM ring
     by quick copies, fast approximate reciprocals, and one K=2 select
     matmul per pair that broadcasts both heads' reciprocal rows.
  D) The out projection is interleaved one matmul at a time into the
     attention chunk stream of the following query tile (a small state
     machine pumps it), filling PE slack while ACT paces the exps.
"""

import numpy as np

import concourse.bass as bass
import concourse.mybir as mybir
import concourse.tile as tile_mod

# ----------------------------------------------------------------------------
# Workaround for this walrus build rejecting instructions that carry more than
# MAX_WAITS semaphore waits ("Too many sync wait commands" in CoreV3GenImpl
# setupSyncWait — hit on Drain and Matmult/S3_LW encodings). Split excess
# waits onto single-wait NOP carrier instructions emitted just before the
# original instruction on the same engine: program order on the sequencer
# makes this semantically identical (waits are AND conditions).
# ----------------------------------------------------------------------------
_MAX_WAITS = 1
_orig_add_instruction = tile_mod.TileContext._add_instruction
_carrier_id = [0]


def _split_waits_add_instruction(self, inst):
    si = inst.sync_info
    if (
        si is not None
        and si.on_wait
        and len(si.on_wait) > _MAX_WAITS
        and inst.engine != mybir.EngineType.Unassigned
    ):
        waits = list(si.on_wait)
        keep = waits[-_MAX_WAITS:]
        for w in waits[:-_MAX_WAITS]:
            _carrier_id[0] += 1
            nop = mybir.InstNoOp(name=f"I-waitc-{_carrier_id[0]}")
            nop.engine = inst.engine
            nop.sync_info = mybir.SyncInfo(on_wait=[w], on_update=[])
            _orig_add_instruction(self, nop)
        inst.sync_info = mybir.SyncInfo(
            on_wait=keep,
            on_update=list(si.on_update) if si.on_update else [],
        )
    _orig_add_instruction(self, inst)


tile_mod.TileContext._add_instruction = _split_waits_add_instruction

from concourse.vector_clock import ScopedClock


def _patched_drain_and_barrier(self, tick_clock, wait_clock):
    # Same wait-splitting for the TileContext exit drain, which is emitted
    # after lowering (outside _add_instruction).
    nc = self.nc
    carrier = nc.sync.nop(nofuse=True)
    wait_clock.add_sem_waits(carrier.ins, ScopedClock({None: tick_clock.global_clock}))
    si = carrier.ins.sync_info
    waits = list(si.on_wait) if si is not None and si.on_wait else []
    if len(waits) > _MAX_WAITS:
        carrier.ins.sync_info = mybir.SyncInfo(
            on_wait=waits[:_MAX_WAITS],
            on_update=list(si.on_update) if si.on_update else [],
        )
        rest = waits[_MAX_WAITS:]
        while rest:
            extra = nc.sync.nop(nofuse=True)
            extra.ins.sync_info = mybir.SyncInfo(
                on_wait=rest[:_MAX_WAITS], on_update=[])
            rest = rest[_MAX_WAITS:]

    nc.sync.drain()
    nc.all_engine_barrier()
    assert self.sems is not None
    popped = nc._tile_sem_poison_stack.pop()
    assert popped is self._sem_poison
    nc.clear_and_free_semaphores(list(self.sems.allocated().values()))
    nc.all_engine_barrier()


tile_mod.TileContext._drain_and_barrier = _patched_drain_and_barrier

# ----------------------------------------------------------------------------

F32 = mybir.dt.float32
F32R = mybir.dt.float32r
BF16 = mybir.dt.bfloat16
ALU = mybir.AluOpType
ACT_EXP = mybir.ActivationFunctionType.Exp
ACT_SQRT = mybir.ActivationFunctionType.Sqrt
ACT_IDENT = mybir.ActivationFunctionType.Identity
U32 = mybir.dt.uint32
ONE_BITS = int(np.float32(1.0).view(np.uint32))

B = 4
TOK = 2048
DIM = 1024
HEADS = 16
DH = 64
HEADS_PC = 8          # heads per core
INNER_PC = HEADS_PC * DH  # 512
NPAIRS = HEADS_PC // 2
QT = 512              # query tile (matmul moving dim)
KC = 128              # key-token chunk (contraction tile)
NKD = DIM // 128      # d-contraction chunks
NQT = TOK // QT       # query tiles
NTT = TOK // KC       # 128-token tiles
EPS = 1e-5
SCALE = DH ** -0.5


def build_program():
    nc = bass.Bass()
    xT = nc.declare_dram_parameter("xT", [DIM, TOK], BF16, isOutput=False)
    w = nc.declare_dram_parameter("w", [DIM, 3 * INNER_PC], BF16, isOutput=False)
    wo = nc.declare_dram_parameter("wo", [INNER_PC, DIM], BF16, isOutput=False)
    masks = nc.declare_dram_parameter("masks", [4, KC, QT], BF16, isOutput=False)
    betaqk = nc.declare_dram_parameter("betaqk", [128, 2 * NPAIRS], F32,
                                       isOutput=False)
    betav = nc.declare_dram_parameter("betav", [1, INNER_PC], F32R,
                                      isOutput=False)
    selm = nc.declare_dram_parameter("selm", [2, 128], BF16, isOutput=False)
    out = nc.declare_dram_parameter("out", [TOK, DIM], F32, isOutput=True)

    import contextlib
    with tile_mod.TileContext(nc) as tc, nc.allow_low_precision(
            "bf16 operand tiles; all matmul accumulation stays fp32 PSUM"):
        with contextlib.ExitStack() as _st:
            const = _st.enter_context(tc.tile_pool(name="const", bufs=1))
            xtp = _st.enter_context(tc.tile_pool(name="xt", bufs=NKD))
            wsp = _st.enter_context(tc.tile_pool(name="wsb", bufs=NKD))
            wop = _st.enter_context(tc.tile_pool(name="wo_sb", bufs=NPAIRS))
            vpool = _st.enter_context(tc.tile_pool(name="vpool", bufs=NTT))
            qkp = _st.enter_context(tc.tile_pool(name="qkT", bufs=NPAIRS))
            otp = _st.enter_context(tc.tile_pool(name="oT", bufs=NPAIRS))
            bcp = _st.enter_context(tc.tile_pool(name="bc", bufs=NQT))
            # ---------------- constants + full prefetch ----------------
            ones_col = const.tile([128, 1], BF16, tag="ones_col")
            nc.vector.memset(ones_col, 1.0)
            ones_row = const.tile([1, 128], F32R, tag="ones_row")
            nc.vector.memset(ones_row.bitcast(U32), ONE_BITS)
            inv_bf = const.tile([1, 128], BF16, tag="inv_bf")
            nc.vector.memset(inv_bf, 1.0 / DIM)
            eps128 = const.tile([128, 1], F32, tag="eps")
            nc.vector.memset(eps128, EPS)
            bqk_sb = const.tile([128, 2 * NPAIRS], F32, tag="bqk")
            nc.sync.dma_start(out=bqk_sb, in_=betaqk[:, :])
            # head-select matrix for the two-head reciprocal broadcast
            sel_sb = const.tile([2, 128], BF16, tag="sel")
            nc.sync.dma_start(out=sel_sb, in_=selm[:, :])
            bv_row = const.tile([1, INNER_PC], F32R, tag="bv_row")
            nc.sync.dma_start(out=bv_row, in_=betav[:, :])
            mask_sb = const.tile([KC, 4, QT], BF16, tag="mask")
            for m in range(4):
                nc.sync.dma_start(out=mask_sb[:, m, :], in_=masks[m, :, :])

            xt = []
            for kc in range(NKD):
                t = xtp.tile([128, TOK], BF16, tag="xt")
                nc.sync.dma_start(out=t, in_=xT[kc * 128:(kc + 1) * 128, :])
                xt.append(t)
            wsb = []
            for kc in range(NKD):
                t = wsp.tile([128, 3 * INNER_PC], BF16, tag="wsb")
                nc.sync.dma_start(out=t, in_=w[kc * 128:(kc + 1) * 128, :])
                wsb.append(t)
            wos = []
            for p in range(NPAIRS):
                t = wop.tile([128, DIM], BF16, tag="wo_sb")
                nc.sync.dma_start(out=t, in_=wo[p * 128:(p + 1) * 128, :])
                wos.append(t)

            # persistent per-pair tiles
            qT = [qkp.tile([128, TOK], BF16, tag="qT", name=f"qT{p}")
                  for p in range(NPAIRS)]
            kT = [qkp.tile([128, TOK], BF16, tag="kT", name=f"kT{p}")
                  for p in range(NPAIRS)]
            oT = [otp.tile([128, TOK], BF16, tag="oT", name=f"oT{p}")
                  for p in range(NPAIRS)]
            bv_bc = const.tile([128, INNER_PC], F32, tag="bv_bc")

            # ---------------- phase A: LN stats + xn ----------------
            sum_r = const.tile([1, TOK], BF16, tag="sum_r")
            sq_r = const.tile([1, TOK], BF16, tag="sq_r")
            with (
                tc.tile_pool(name="sqp", bufs=3) as sqp,
            ):
                with tc.tile_pool(name="ps_stats", bufs=1,
                                  space="PSUM") as pstat:
                    sum_ps = pstat.tile([1, TOK], F32, tag="sum")
                    sq_ps = pstat.tile([1, TOK], F32, tag="sq")
                    for kc in range(NKD):
                        for nt in range(NQT):
                            sl = slice(nt * QT, (nt + 1) * QT)
                            sq_t = sqp.tile([128, QT], BF16, tag="sq_t")
                            nc.vector.tensor_mul(sq_t, xt[kc][:, sl],
                                                 xt[kc][:, sl])
                            nc.tensor.matmul(
                                out=sum_ps[0:1, sl], lhsT=ones_col,
                                rhs=xt[kc][:, sl],
                                start=(kc == 0), stop=(kc == NKD - 1))
                            nc.tensor.matmul(
                                out=sq_ps[0:1, sl], lhsT=ones_col,
                                rhs=sq_t,
                                start=(kc == 0), stop=(kc == NKD - 1))
                    for nt in range(NQT):
                        sl = slice(nt * QT, (nt + 1) * QT)
                        nc.vector.tensor_copy(sum_r[0:1, sl], sum_ps[0:1, sl])
                        nc.scalar.copy(sq_r[0:1, sl], sq_ps[0:1, sl])

                # broadcast sum/sumsq rows (with the 1/D fold) to [128, QT]
                # and run the whole stats chain on wide tiles
                bc_rstd, bc_nmr = [], []
                with (
                    tc.tile_pool(name="ps_bc", bufs=4, space="PSUM") as pbc,
                    tc.tile_pool(name="stw", bufs=2) as stw,
                ):
                    for nt in range(NQT):
                        sl = slice(nt * QT, (nt + 1) * QT)
                        mu_ps = pbc.tile([128, QT], F32, tag="bc_ps")
                        nc.tensor.matmul(out=mu_ps, lhsT=inv_bf,
                                         rhs=sum_r[0:1, sl],
                                         start=True, stop=True)
                        ex_ps = pbc.tile([128, QT], F32, tag="bc_ps")
                        nc.tensor.matmul(out=ex_ps, lhsT=inv_bf,
                                         rhs=sq_r[0:1, sl],
                                         start=True, stop=True)
                        var = stw.tile([128, QT], F32, tag="var")
                        nc.scalar.square(var, mu_ps)
                        nc.vector.tensor_sub(var, ex_ps, var)
                        nc.scalar.activation(out=var, in_=var, func=ACT_SQRT,
                                             bias=eps128, scale=1.0)  # std
                        rstd = stw.tile([128, QT], F32, tag="rstd")
                        nc.vector.reciprocal(rstd, var)
                        br = bcp.tile([128, QT], BF16, tag="bc_rstd")
                        nc.vector.tensor_copy(br, rstd)
                        bc_rstd.append(br)
                        bn = bcp.tile([128, QT], BF16, tag="bc_nmr")
                        nc.vector.tensor_mul(bn, mu_ps, rstd)  # mu*rstd
                        bc_nmr.append(bn)
                    bv_ps = pbc.tile([128, INNER_PC], F32, tag="bv_ps",
                                     bufs=1)
                    nc.tensor.matmul(out=bv_ps, lhsT=ones_row,
                                     rhs=bv_row, start=True, stop=True)
                    nc.vector.tensor_copy(bv_bc, bv_ps)

                # xn = x*rstd - mu*rstd, in place; token-slice-major order so
                # phase B can start on slice 0 early
                for nt in range(NQT):
                    sl = slice(nt * QT, (nt + 1) * QT)
                    for kc in range(NKD):
                        nc.vector.tensor_mul(xt[kc][:, sl], xt[kc][:, sl],
                                             bc_rstd[nt])
                        nc.vector.tensor_sub(xt[kc][:, sl], xt[kc][:, sl],
                                             bc_nmr[nt])

            # ---------------- phase B: V then Q^T/K^T projections ----------
            v_sb = []
            with (
                tc.tile_pool(name="ps_v", bufs=2, space="PSUM") as psv,
                tc.tile_pool(name="ps_qk", bufs=3, space="PSUM") as psqk,
            ):
                for tt in range(NTT):
                    tsl = slice(tt * KC, (tt + 1) * KC)
                    v_ps = psv.tile([128, INNER_PC], F32, tag="v_ps")
                    for kc in range(NKD):
                        nc.tensor.matmul(
                            out=v_ps, lhsT=xt[kc][:, tsl],
                            rhs=wsb[kc][:, 2 * INNER_PC:3 * INNER_PC],
                            start=(kc == 0), stop=(kc == NKD - 1))
                    vt = vpool.tile([128, HEADS_PC * (DH + 1)], BF16, tag="v_sb")
                    v3 = vt.rearrange("p (h w) -> p h w", w=DH + 1)
                    nc.vector.tensor_add(
                        v3[:, :, 0:DH],
                        v_ps.rearrange("p (h w) -> p h w", w=DH),
                        bv_bc.rearrange("p (h w) -> p h w", w=DH))
                    nc.vector.memset(v3[:, :, DH:DH + 1], 1.0)
                    v_sb.append(vt)

                for p in range(NPAIRS):
                    for di, (dst, cofs) in enumerate(
                        ((qT[p], p * 128), (kT[p], INNER_PC + p * 128)),
                    ):
                        for nt in range(NQT):
                            sl = slice(nt * QT, (nt + 1) * QT)
                            ps = psqk.tile([128, QT], F32, tag="qk_ps")
                            for kc in range(NKD):
                                nc.tensor.matmul(
                                    out=ps,
                                    lhsT=wsb[kc][:, cofs:cofs + 128],
                                    rhs=xt[kc][:, sl],
                                    start=(kc == 0), stop=(kc == NKD - 1))
                            nc.scalar.activation(
                                out=dst[:, sl], in_=ps, func=ACT_IDENT,
                                bias=bqk_sb[:, 2 * p + di:2 * p + di + 1],
                                scale=1.0)

            # -------- phases C+D: attention with interleaved out-proj -------
            with contextlib.ExitStack() as _stC:
                pss = _stC.enter_context(
                    tc.tile_pool(name="ps_s", bufs=2, space="PSUM"))
                pso = _stC.enter_context(
                    tc.tile_pool(name="ps_o", bufs=3, space="PSUM"))
                pmisc = _stC.enter_context(
                    tc.tile_pool(name="ps_misc", bufs=1, space="PSUM"))
                ppool = _stC.enter_context(tc.tile_pool(name="psb", bufs=4))
                orp = _stC.enter_context(tc.tile_pool(name="orp", bufs=8))
                rbp = _stC.enter_context(tc.tile_pool(name="rbp", bufs=2))
                recp = _stC.enter_context(tc.tile_pool(name="rec", bufs=4))
                denp = _stC.enter_context(tc.tile_pool(name="den", bufs=8))
                outp = _stC.enter_context(tc.tile_pool(name="out_sb", bufs=3))
                # --- out-proj state machine: one matmul per pump step ---
                d_pending = []   # queue of token-128-tile x dim-half units
                d_state = {"ps": None, "step": 0}

                def d_pump(nsteps):
                    for _ in range(nsteps):
                        if not d_pending:
                            return
                        tt, nb = d_pending[0]
                        tsl = slice(tt * KC, (tt + 1) * KC)
                        nsl = slice(nb * QT, (nb + 1) * QT)
                        if d_state["ps"] is None:
                            d_state["ps"] = pmisc.tile([128, QT], F32,
                                                       tag="d_ps",
                                                       name="d_ps")
                            d_state["step"] = 0
                        p_i = d_state["step"]
                        nc.tensor.matmul(
                            out=d_state["ps"], lhsT=oT[p_i][:, tsl],
                            rhs=wos[p_i][:, nsl],
                            start=(p_i == 0), stop=(p_i == NPAIRS - 1))
                        d_state["step"] += 1
                        if d_state["step"] == NPAIRS:
                            ob = outp.tile([128, QT], F32, tag="out_sb")
                            nc.vector.tensor_copy(ob, d_state["ps"])
                            nc.sync.dma_start(out=out[tsl, nsl], in_=ob)
                            d_state["ps"] = None
                            d_pending.pop(0)

                def scores_chunk(p, t_i, c, nch):
                    """Emit scores matmuls + exp + mask for chunk c; returns
                    the p_sb tile and the live q-range start (for PV)."""
                    qsl = slice(t_i * QT, (t_i + 1) * QT)
                    csl = slice(c * KC, (c + 1) * KC)
                    m = c - (nch - 4)
                    lo = 128 * m if m >= 1 else 0   # live q-range start
                    s_ps = pss.tile([128, 2 * QT], F32, tag="s_ps")
                    qlo = slice(t_i * QT + lo, (t_i + 1) * QT)
                    nc.tensor.matmul(
                        out=s_ps[:, lo:QT],
                        lhsT=kT[p][0:DH, csl], rhs=qT[p][0:DH, qlo],
                        start=True, stop=True)
                    nc.tensor.matmul(
                        out=s_ps[:, QT + lo:2 * QT],
                        lhsT=kT[p][DH:128, csl], rhs=qT[p][DH:128, qlo],
                        start=True, stop=True)
                    p_sb = ppool.tile([128, 2 * QT], BF16, tag="p_sb")
                    s3 = s_ps.rearrange("p (h q) -> p h q", q=QT)
                    p3 = p_sb.rearrange("p (h q) -> p h q", q=QT)
                    nc.scalar.activation(out=p3[:, :, lo:QT],
                                         in_=s3[:, :, lo:QT],
                                         func=ACT_EXP, scale=SCALE)
                    if m >= 0:
                        for h in range(2):
                            nc.vector.tensor_mul(
                                p_sb[:, h * QT + lo:(h + 1) * QT],
                                p_sb[:, h * QT + lo:(h + 1) * QT],
                                mask_sb[:, m, lo:QT])
                    return p_sb, lo

                def pv_chunk(p, c, nch, o_ps, p_sb, lo):
                    for h in range(2):
                        hc = (2 * p + h) * (DH + 1)
                        nc.tensor.matmul(
                            out=o_ps[h][:, lo:QT],
                            lhsT=v_sb[c][:, hc:hc + DH + 1],
                            rhs=p_sb[:, h * QT + lo:(h + 1) * QT],
                            start=(c == 0), stop=(c == nch - 1),
                            skip_group_check=True)

                for t_i in range(NQT):
                    qsl = slice(t_i * QT, (t_i + 1) * QT)
                    nch = (t_i + 1) * NQT
                    if t_i > 0:
                        d_pending.extend(
                            ((t_i - 1) * NQT + u, nb)
                            for u in range(NQT) for nb in range(2))
                    den8 = denp.tile([2 * NPAIRS, QT], F32, tag="den8",
                                     bufs=2)
                    o_raws = []
                    for p in range(NPAIRS):
                        o_ps = [pso.tile([DH + 1, QT], F32, tag="o_ps",
                                         name=f"o_ps{h}") for h in range(2)]
                        prev = None
                        for c in range(nch):
                            cur = scores_chunk(p, t_i, c, nch)
                            if prev is not None:
                                pv_chunk(p, c - 1, nch, o_ps, *prev)
                            prev = cur
                            d_pump(1)
                        pv_chunk(p, nch - 1, nch, o_ps, *prev)

                        # quick copies free the o_ps ring: raw O rows (bf16)
                        # and the denominator row (f32, DMA'd into the
                        # per-tile batch for one shared reciprocal)
                        for h in range(2):
                            dr = denp.tile([1, QT], F32, tag="den_row")
                            nc.vector.tensor_copy(dr, o_ps[h][DH:DH + 1, :])
                            nc.sync.dma_start(
                                out=den8[2 * p + h:2 * p + h + 1, :], in_=dr)
                            orh = orp.tile([DH, QT], BF16, tag="o_raw",
                                           name=f"o_raw{h}")
                            nc.vector.tensor_copy(orh, o_ps[h][0:DH, :])
                            o_raws.append(orh)

                    # one batched reciprocal for all 8 denominator rows,
                    # scattered back to partition-0-aligned [2, QT] tiles
                    den8r = denp.tile([2 * NPAIRS, QT], BF16, tag="den8r",
                                      bufs=2)
                    nc.vector.reciprocal(den8r, den8)
                    for p in range(NPAIRS):
                        rec2 = recp.tile([2, QT], BF16, tag="rec2")
                        nc.sync.dma_start(out=rec2,
                                          in_=den8r[2 * p:2 * p + 2, :])
                        rb_ps = pso.tile([128, QT], F32, tag="o_ps",
                                         name="rb_ps")
                        nc.tensor.matmul(out=rb_ps, lhsT=sel_sb, rhs=rec2,
                                         start=True, stop=True)
                        for h in range(2):
                            rbh = rbp.tile([DH, QT], BF16, tag="rb",
                                           name=f"rb{h}")
                            nc.vector.tensor_copy(
                                rbh, rb_ps[h * DH:(h + 1) * DH, :])
                            nc.vector.tensor_mul(
                                oT[p][h * DH:(h + 1) * DH, qsl],
                                o_raws[2 * p + h], rbh)
                    d_pump(2 * NQT)  # finish any stragglers for this tile
                # trailing out-proj for the last query tile
                d_pending.extend(
                    ((NQT - 1) * NQT + u, nb)
                    for u in range(NQT) for nb in range(2))
                d_pump(len(d_pending) * NPAIRS)

    return nc


def make_masks():
    import ml_dtypes

    j = np.arange(KC)[:, None]
    i = np.arange(QT)[None, :]
    return np.stack(
        [(i >= j + 128 * m) for m in range(4)]).astype(ml_dtypes.bfloat16)


def make_in_maps(x, ln_gamma, ln_beta, w_qkv, w_out):
    import ml_dtypes

    bf16 = ml_dtypes.bfloat16
    x = np.asarray(x, np.float32)
    g_ = np.asarray(ln_gamma, np.float32)
    b_ = np.asarray(ln_beta, np.float32)
    w_qkv = np.asarray(w_qkv, np.float32)
    w_out = np.asarray(w_out, np.float32)
    masks = make_masks()
    in_maps = []
    for c in range(8):
        b = c // 2
        g = c % 2
        cs = slice(g * INNER_PC, (g + 1) * INNER_PC)
        Wraw = np.concatenate(
            [w_qkv[:, 0 * DIM:1 * DIM][:, cs],
             w_qkv[:, 1 * DIM:2 * DIM][:, cs],
             w_qkv[:, 2 * DIM:3 * DIM][:, cs]], axis=1)
        Wp = Wraw * g_[:, None]
        bqkv = b_ @ Wraw  # [3*INNER_PC]
        betaqk = np.empty((128, 2 * NPAIRS), np.float32)
        for p in range(NPAIRS):
            betaqk[:, 2 * p] = bqkv[p * 128:(p + 1) * 128]
            betaqk[:, 2 * p + 1] = bqkv[INNER_PC + p * 128:
                                        INNER_PC + (p + 1) * 128]
        betav = np.ascontiguousarray(
            bqkv[2 * INNER_PC:3 * INNER_PC][None, :]).astype(np.float32)
        selm = np.zeros((2, 128), np.float32)
        selm[0, 0:64] = 1.0
        selm[1, 64:128] = 1.0
        in_maps.append({
            "xT": np.ascontiguousarray(x[b].T).astype(bf16),
            "w": Wp.astype(bf16),
            "wo": np.ascontiguousarray(w_out[cs, :]).astype(bf16),
            "masks": masks,
            "betaqk": betaqk,
            "betav": betav,
            "selm": selm.astype(bf16),
        })
    return in_maps


_PROG = None


def kernel(x, ln_gamma, ln_beta, w_qkv, w_out):
    global _PROG
    from concourse.bass_utils import run_bass_kernel_spmd

    if _PROG is None:
        _PROG = build_program()
    in_maps = make_in_maps(x, ln_gamma, ln_beta, w_qkv, w_out)
    res = run_bass_kernel_spmd(_PROG, in_maps, list(range(8)))
    parts = [res.results[c]["out"] for c in range(8)]
    out = np.empty((B, TOK, DIM), np.float32)
    for b in range(B):
        out[b] = parts[2 * b] + parts[2 * b + 1]
    return out


# revision 32
# speedup vs baseline: 1.8061x; 1.0362x over previous
"""Causal attention block (LN -> QKV -> causal MHA -> out-proj) on 8 trn2
NeuronCores via Bass/Tile.

Sharding: core c handles batch b=c//2 and head-group g=c%2 (8 of 16 heads).
Data parallel over batch, tensor parallel over heads; the out-proj partial
sums (2 per batch) are reduced on the host during the gather, so the device
program needs no collectives and is pure SPMD.

Per-core pipeline (all matmul operands bf16, fp32 PSUM accumulation):
  A) x^T arrives host-transposed d-major in bf16. LN stats via ones-column
     matmuls into [1, 2048] PSUM rows; the rows are PE-broadcast (with the
     1/D mean division folded into the broadcast operand) to [128, 512]
     tiles and the whole mu/var/rstd postprocess runs on those wide tiles.
     xn = x*rstd - mu*rstd is materialized in place (2 DVE ops per tile,
     earliest token slice first so phase B can start), which removes every
     seed matmul from the projections.
  B) V (token-major, with a 65th ones column for the softmax denominator)
     and Q^T/K^T for all 4 head pairs. QK PSUM->SBUF copies run on the ACT
     engine as Identity-with-bias (applies the LN beta term for free).
  C) Attention per query tile x head pair, software-pipelined so the PE
     never waits on the ACT exp: scores of chunk c+1 are emitted before
     P@V of chunk c. Diagonal chunks compute/exp/mask only the causally
     live [128m:512] sub-rectangle. The softmax denominator rides as a
     65th ones-column of V; normalization is decoupled from the PSUM ring
     by quick copies, fast approximate reciprocals, and one K=2 select
     matmul per pair that broadcasts both heads' reciprocal rows.
  D) The out projection is interleaved one matmul at a time into the
     attention chunk stream of the following query tile (a small state
     machine pumps it), filling PE slack while ACT paces the exps.
"""

import numpy as np

import concourse.bass as bass
import concourse.mybir as mybir
import concourse.tile as tile_mod

# ----------------------------------------------------------------------------
# Workaround for this walrus build rejecting instructions that carry more than
# MAX_WAITS semaphore waits ("Too many sync wait commands" in CoreV3GenImpl
# setupSyncWait — hit on Drain and Matmult/S3_LW encodings). Split excess
# waits onto single-wait NOP carrier instructions emitted just before the
# original instruction on the same engine: program order on the sequencer
# makes this semantically identical (waits are AND conditions).
# ----------------------------------------------------------------------------
_MAX_WAITS = 1
_orig_add_instruction = tile_mod.TileContext._add_instruction
_carrier_id = [0]


def _split_waits_add_instruction(self, inst):
    si = inst.sync_info
    if (
        si is not None
        and si.on_wait
        and len(si.on_wait) > _MAX_WAITS
        and inst.engine != mybir.EngineType.Unassigned
    ):
        waits = list(si.on_wait)
        keep = waits[-_MAX_WAITS:]
        for w in waits[:-_MAX_WAITS]:
            _carrier_id[0] += 1
            nop = mybir.InstNoOp(name=f"I-waitc-{_carrier_id[0]}")
            nop.engine = inst.engine
            nop.sync_info = mybir.SyncInfo(on_wait=[w], on_update=[])
            _orig_add_instruction(self, nop)
        inst.sync_info = mybir.SyncInfo(
            on_wait=keep,
            on_update=list(si.on_update) if si.on_update else [],
        )
    _orig_add_instruction(self, inst)


tile_mod.TileContext._add_instruction = _split_waits_add_instruction

from concourse.vector_clock import ScopedClock


def _patched_drain_and_barrier(self, tick_clock, wait_clock):
    # Same wait-splitting for the TileContext exit drain, which is emitted
    # after lowering (outside _add_instruction).
    nc = self.nc
    carrier = nc.sync.nop(nofuse=True)
    wait_clock.add_sem_waits(carrier.ins, ScopedClock({None: tick_clock.global_clock}))
    si = carrier.ins.sync_info
    waits = list(si.on_wait) if si is not None and si.on_wait else []
    if len(waits) > _MAX_WAITS:
        carrier.ins.sync_info = mybir.SyncInfo(
            on_wait=waits[:_MAX_WAITS],
            on_update=list(si.on_update) if si.on_update else [],
        )
        rest = waits[_MAX_WAITS:]
        while rest:
            extra = nc.sync.nop(nofuse=True)
            extra.ins.sync_info = mybir.SyncInfo(
                on_wait=rest[:_MAX_WAITS], on_update=[])
            rest = rest[_MAX_WAITS:]

    nc.sync.drain()
    nc.all_engine_barrier()
    assert self.sems is not None
    popped = nc._tile_sem_poison_stack.pop()
    assert popped is self._sem_poison
    nc.clear_and_free_semaphores(list(self.sems.allocated().values()))
    nc.all_engine_barrier()


tile_mod.TileContext._drain_and_barrier = _patched_drain_and_barrier

# ----------------------------------------------------------------------------

F32 = mybir.dt.float32
F32R = mybir.dt.float32r
BF16 = mybir.dt.bfloat16
ALU = mybir.AluOpType
ACT_EXP = mybir.ActivationFunctionType.Exp
ACT_SQRT = mybir.ActivationFunctionType.Sqrt
ACT_IDENT = mybir.ActivationFunctionType.Identity
U32 = mybir.dt.uint32
ONE_BITS = int(np.float32(1.0).view(np.uint32))

B = 4
TOK = 2048
DIM = 1024
HEADS = 16
DH = 64
HEADS_PC = 8          # heads per core
INNER_PC = HEADS_PC * DH  # 512
NPAIRS = HEADS_PC // 2
QT = 512              # query tile (matmul moving dim)
KC = 128              # key-token chunk (contraction tile)
NKD = DIM // 128      # d-contraction chunks
NQT = TOK // QT       # query tiles
NTT = TOK // KC       # 128-token tiles
EPS = 1e-5
SCALE = DH ** -0.5


def build_program():
    nc = bass.Bass()
    xT = nc.declare_dram_parameter("xT", [DIM, TOK], BF16, isOutput=False)
    w = nc.declare_dram_parameter("w", [DIM, 3 * INNER_PC], BF16, isOutput=False)
    wo = nc.declare_dram_parameter("wo", [INNER_PC, DIM], BF16, isOutput=False)
    masks = nc.declare_dram_parameter("masks", [4, KC, QT], BF16, isOutput=False)
    betaqk = nc.declare_dram_parameter("betaqk", [128, 2 * NPAIRS], F32,
                                       isOutput=False)
    betav = nc.declare_dram_parameter("betav", [1, INNER_PC], F32R,
                                      isOutput=False)
    selm = nc.declare_dram_parameter("selm", [2, 128], BF16, isOutput=False)
    out = nc.declare_dram_parameter("out", [TOK, DIM], F32, isOutput=True)

    import contextlib
    with tile_mod.TileContext(nc) as tc, nc.allow_low_precision(
            "bf16 operand tiles; all matmul accumulation stays fp32 PSUM"):
        with contextlib.ExitStack() as _st:
            const = _st.enter_context(tc.tile_pool(name="const", bufs=1))
            wop = _st.enter_context(tc.tile_pool(name="wo_sb", bufs=NPAIRS))
            vpool = _st.enter_context(tc.tile_pool(name="vpool", bufs=NTT))
            qkp = _st.enter_context(tc.tile_pool(name="qkT", bufs=NPAIRS))
            otp = _st.enter_context(tc.tile_pool(name="oT", bufs=NPAIRS))
            bcp = _st.enter_context(tc.tile_pool(name="bc", bufs=NQT))
            # ---------------- constants + full prefetch ----------------
            ones_col = const.tile([128, 1], BF16, tag="ones_col")
            nc.vector.memset(ones_col, 1.0)
            ones_row = const.tile([1, 128], F32R, tag="ones_row")
            nc.vector.memset(ones_row.bitcast(U32), ONE_BITS)
            inv_bf = const.tile([1, 128], BF16, tag="inv_bf")
            nc.vector.memset(inv_bf, 1.0 / DIM)
            eps128 = const.tile([128, 1], F32, tag="eps")
            nc.vector.memset(eps128, EPS)
            bqk_sb = const.tile([128, 2 * NPAIRS], F32, tag="bqk")
            nc.sync.dma_start(out=bqk_sb, in_=betaqk[:, :])
            # head-select matrix for the two-head reciprocal broadcast
            sel_sb = const.tile([2, 128], BF16, tag="sel")
            nc.sync.dma_start(out=sel_sb, in_=selm[:, :])
            bv_row = const.tile([1, INNER_PC], F32R, tag="bv_row")
            nc.sync.dma_start(out=bv_row, in_=betav[:, :])
            mask_sb = const.tile([KC, 4, QT], BF16, tag="mask")
            for m in range(4):
                nc.sync.dma_start(out=mask_sb[:, m, :], in_=masks[m, :, :])

            _stAB = contextlib.ExitStack()
            xtp = _stAB.enter_context(tc.tile_pool(name="xt", bufs=NKD))
            wsp = _stAB.enter_context(tc.tile_pool(name="wsb", bufs=NKD))
            xt = []
            for kc in range(NKD):
                t = xtp.tile([128, TOK], BF16, tag="xt")
                nc.sync.dma_start(out=t, in_=xT[kc * 128:(kc + 1) * 128, :])
                xt.append(t)
            wsb = []
            for kc in range(NKD):
                t = wsp.tile([128, 3 * INNER_PC], BF16, tag="wsb")
                nc.sync.dma_start(out=t, in_=w[kc * 128:(kc + 1) * 128, :])
                wsb.append(t)
            wos = []
            for p in range(NPAIRS):
                t = wop.tile([128, DIM], BF16, tag="wo_sb")
                nc.sync.dma_start(out=t, in_=wo[p * 128:(p + 1) * 128, :])
                wos.append(t)

            # persistent per-pair tiles
            qT = [qkp.tile([128, TOK], BF16, tag="qT", name=f"qT{p}")
                  for p in range(NPAIRS)]
            kT = [qkp.tile([128, TOK], BF16, tag="kT", name=f"kT{p}")
                  for p in range(NPAIRS)]
            oT = [otp.tile([128, TOK], BF16, tag="oT", name=f"oT{p}")
                  for p in range(NPAIRS)]
            bv_bc = const.tile([128, INNER_PC], F32, tag="bv_bc")

            # ---------------- phase A: LN stats + xn ----------------
            with (
                tc.tile_pool(name="sqp", bufs=3) as sqp,
            ):
                sum_r = sqp.tile([1, TOK], F32, tag="sum_r", bufs=1)
                sq_r = sqp.tile([1, TOK], F32, tag="sq_r", bufs=1)
                with tc.tile_pool(name="ps_stats", bufs=1,
                                  space="PSUM") as pstat:
                    sum_ps = pstat.tile([1, TOK], F32, tag="sum")
                    sq_ps = pstat.tile([1, TOK], F32, tag="sq")
                    for kc in range(NKD):
                        for nt in range(NQT):
                            sl = slice(nt * QT, (nt + 1) * QT)
                            sq_t = sqp.tile([128, QT], BF16, tag="sq_t")
                            nc.vector.tensor_mul(sq_t, xt[kc][:, sl],
                                                 xt[kc][:, sl])
                            nc.tensor.matmul(
                                out=sum_ps[0:1, sl], lhsT=ones_col,
                                rhs=xt[kc][:, sl],
                                start=(kc == 0), stop=(kc == NKD - 1))
                            nc.tensor.matmul(
                                out=sq_ps[0:1, sl], lhsT=ones_col,
                                rhs=sq_t,
                                start=(kc == 0), stop=(kc == NKD - 1))
                    for nt in range(NQT):
                        sl = slice(nt * QT, (nt + 1) * QT)
                        nc.vector.tensor_copy(sum_r[0:1, sl], sum_ps[0:1, sl])
                        nc.scalar.copy(sq_r[0:1, sl], sq_ps[0:1, sl])

                # compact [128, 16] stats chain (partition-parallel, so the
                # expensive reciprocal runs on 16 elements per lane), then
                # DMA-reshape back to f32r rows for the PE broadcasts
                bc_rstd, bc_nmr = [], []
                with (
                    tc.tile_pool(name="ps_bc", bufs=4, space="PSUM") as pbc,
                    tc.tile_pool(name="stw", bufs=1) as stw,
                ):
                    CW = TOK // 128  # 16
                    sum_c = stw.tile([128, CW], F32, tag="sum_c")
                    nc.sync.dma_start(out=sum_c, in_=sum_r[0:1, :])
                    sq_c = stw.tile([128, CW], F32, tag="sq_c")
                    nc.sync.dma_start(out=sq_c, in_=sq_r[0:1, :])
                    mu_c = stw.tile([128, CW], F32, tag="mu_c")
                    nc.vector.tensor_scalar_mul(mu_c, sum_c, 1.0 / DIM)
                    var_c = stw.tile([128, CW], F32, tag="var_c")
                    nc.scalar.square(var_c, mu_c)
                    ex_c = stw.tile([128, CW], F32, tag="ex_c")
                    nc.vector.tensor_scalar_mul(ex_c, sq_c, 1.0 / DIM)
                    nc.vector.tensor_sub(var_c, ex_c, var_c)
                    nc.scalar.activation(out=var_c, in_=var_c, func=ACT_SQRT,
                                         bias=eps128, scale=1.0)  # std
                    rstd_c = stw.tile([128, CW], F32R, tag="rstd_c")
                    nc.vector.reciprocal(rstd_c, var_c)
                    nmr_c = stw.tile([128, CW], F32R, tag="nmr_c")
                    nc.vector.tensor_mul(nmr_c, mu_c, rstd_c)  # mu*rstd
                    rstd_row = stw.tile([1, TOK], F32R, tag="rstd_row")
                    nc.sync.dma_start(out=rstd_row[0:1, :], in_=rstd_c)
                    nmr_row = stw.tile([1, TOK], F32R, tag="nmr_row")
                    nc.sync.dma_start(out=nmr_row[0:1, :], in_=nmr_c)
                    for src_row, dstl, tg in ((rstd_row, bc_rstd, "bc_rstd"),
                                              (nmr_row, bc_nmr, "bc_nmr")):
                        for nt in range(NQT):
                            sl = slice(nt * QT, (nt + 1) * QT)
                            ps = pbc.tile([128, QT], F32, tag="bc_ps",
                                          name="ps")
                            nc.tensor.matmul(out=ps, lhsT=ones_row,
                                             rhs=src_row[0:1, sl],
                                             start=True, stop=True)
                            t = bcp.tile([128, QT], BF16, tag=tg, name="t")
                            nc.vector.tensor_copy(t, ps)
                            dstl.append(t)
                    bv_ps = pbc.tile([128, INNER_PC], F32, tag="bv_ps",
                                     bufs=1)
                    nc.tensor.matmul(out=bv_ps, lhsT=ones_row,
                                     rhs=bv_row, start=True, stop=True)
                    nc.vector.tensor_copy(bv_bc, bv_ps)

                # xn = x*rstd - mu*rstd, in place; token-slice-major order so
                # phase B can start on slice 0 early
                for nt in range(NQT):
                    sl = slice(nt * QT, (nt + 1) * QT)
                    for kc in range(NKD):
                        nc.vector.tensor_mul(xt[kc][:, sl], xt[kc][:, sl],
                                             bc_rstd[nt])
                        nc.vector.tensor_sub(xt[kc][:, sl], xt[kc][:, sl],
                                             bc_nmr[nt])

            # ---------------- phase B: V then Q^T/K^T projections ----------
            v_sb = []
            with (
                tc.tile_pool(name="ps_v", bufs=2, space="PSUM") as psv,
                tc.tile_pool(name="ps_qk", bufs=3, space="PSUM") as psqk,
            ):
                for tt in range(NTT):
                    tsl = slice(tt * KC, (tt + 1) * KC)
                    v_ps = psv.tile([128, INNER_PC], F32, tag="v_ps")
                    for kc in range(NKD):
                        nc.tensor.matmul(
                            out=v_ps, lhsT=xt[kc][:, tsl],
                            rhs=wsb[kc][:, 2 * INNER_PC:3 * INNER_PC],
                            start=(kc == 0), stop=(kc == NKD - 1))
                    vt = vpool.tile([128, HEADS_PC * (DH + 1)], BF16, tag="v_sb")
                    v3 = vt.rearrange("p (h w) -> p h w", w=DH + 1)
                    nc.vector.tensor_add(
                        v3[:, :, 0:DH],
                        v_ps.rearrange("p (h w) -> p h w", w=DH),
                        bv_bc.rearrange("p (h w) -> p h w", w=DH))
                    nc.vector.memset(v3[:, :, DH:DH + 1], 1.0)
                    v_sb.append(vt)

                for p in range(NPAIRS):
                    for di, (dst, cofs) in enumerate(
                        ((qT[p], p * 128), (kT[p], INNER_PC + p * 128)),
                    ):
                        for nt in range(NQT):
                            sl = slice(nt * QT, (nt + 1) * QT)
                            ps = psqk.tile([128, QT], F32, tag="qk_ps")
                            for kc in range(NKD):
                                nc.tensor.matmul(
                                    out=ps,
                                    lhsT=wsb[kc][:, cofs:cofs + 128],
                                    rhs=xt[kc][:, sl],
                                    start=(kc == 0), stop=(kc == NKD - 1))
                            nc.scalar.activation(
                                out=dst[:, sl], in_=ps, func=ACT_IDENT,
                                bias=bqk_sb[:, 2 * p + di:2 * p + di + 1],
                                scale=1.0)

            _stAB.close()

            # -------- phases C+D: attention with interleaved out-proj -------
            with contextlib.ExitStack() as _stC:
                pss = _stC.enter_context(
                    tc.tile_pool(name="ps_s", bufs=2, space="PSUM"))
                pso = _stC.enter_context(
                    tc.tile_pool(name="ps_o", bufs=2, space="PSUM"))
                prb = _stC.enter_context(
                    tc.tile_pool(name="ps_rb", bufs=1, space="PSUM"))
                pmisc = _stC.enter_context(
                    tc.tile_pool(name="ps_misc", bufs=1, space="PSUM"))
                ppool = _stC.enter_context(tc.tile_pool(name="psb", bufs=4))
                orp = _stC.enter_context(tc.tile_pool(name="orp", bufs=16))
                rbp = _stC.enter_context(tc.tile_pool(name="rbp", bufs=2))
                recp = _stC.enter_context(tc.tile_pool(name="rec", bufs=4))
                denp = _stC.enter_context(tc.tile_pool(name="den", bufs=8))
                outp = _stC.enter_context(tc.tile_pool(name="out_sb", bufs=3))
                # --- out-proj state machine: one matmul per pump step ---
                d_pending = []   # queue of token-128-tile x dim-half units
                d_state = {"ps": None, "step": 0}

                def d_pump(nsteps):
                    for _ in range(nsteps):
                        if not d_pending:
                            return
                        tt, nb = d_pending[0]
                        tsl = slice(tt * KC, (tt + 1) * KC)
                        nsl = slice(nb * QT, (nb + 1) * QT)
                        if d_state["ps"] is None:
                            d_state["ps"] = pmisc.tile([128, QT], F32,
                                                       tag="d_ps",
                                                       name="d_ps")
                            d_state["step"] = 0
                        p_i = d_state["step"]
                        nc.tensor.matmul(
                            out=d_state["ps"], lhsT=oT[p_i][:, tsl],
                            rhs=wos[p_i][:, nsl],
                            start=(p_i == 0), stop=(p_i == NPAIRS - 1))
                        d_state["step"] += 1
                        if d_state["step"] == NPAIRS:
                            ob = outp.tile([128, QT], F32, tag="out_sb")
                            nc.vector.tensor_copy(ob, d_state["ps"])
                            nc.sync.dma_start(out=out[tsl, nsl], in_=ob)
                            d_state["ps"] = None
                            d_pending.pop(0)

                def scores_chunk(p, t_i, c, nch):
                    """Emit scores matmuls + exp + mask for chunk c; returns
                    the p_sb tile and the live q-range start (for PV)."""
                    qsl = slice(t_i * QT, (t_i + 1) * QT)
                    csl = slice(c * KC, (c + 1) * KC)
                    m = c - (nch - 4)
                    lo = 128 * m if m >= 1 else 0   # live q-range start
                    s_ps = pss.tile([128, 2 * QT], F32, tag="s_ps")
                    qlo = slice(t_i * QT + lo, (t_i + 1) * QT)
                    nc.tensor.matmul(
                        out=s_ps[:, lo:QT],
                        lhsT=kT[p][0:DH, csl], rhs=qT[p][0:DH, qlo],
                        start=True, stop=True)
                    nc.tensor.matmul(
                        out=s_ps[:, QT + lo:2 * QT],
                        lhsT=kT[p][DH:128, csl], rhs=qT[p][DH:128, qlo],
                        start=True, stop=True)
                    p_sb = ppool.tile([128, 2 * QT], BF16, tag="p_sb")
                    s3 = s_ps.rearrange("p (h q) -> p h q", q=QT)
                    p3 = p_sb.rearrange("p (h q) -> p h q", q=QT)
                    nc.scalar.activation(out=p3[:, :, lo:QT],
                                         in_=s3[:, :, lo:QT],
                                         func=ACT_EXP, scale=SCALE)
                    if m >= 0:
                        for h in range(2):
                            nc.vector.tensor_mul(
                                p_sb[:, h * QT + lo:(h + 1) * QT],
                                p_sb[:, h * QT + lo:(h + 1) * QT],
                                mask_sb[:, m, lo:QT])
                    return p_sb, lo

                def pv_chunk(p, c, nch, o_ps, p_sb, lo):
                    for h in range(2):
                        hc = (2 * p + h) * (DH + 1)
                        nc.tensor.matmul(
                            out=o_ps[h][:, lo:QT],
                            lhsT=v_sb[c][:, hc:hc + DH + 1],
                            rhs=p_sb[:, h * QT + lo:(h + 1) * QT],
                            start=(c == 0), stop=(c == nch - 1),
                            skip_group_check=True)

                # normalize steps for tile t_i are deferred and pumped
                # into tile t_i+1's chunk stream, ahead of that tile's
                # out-proj units (norms write the oT slices D reads)
                pend_norm = []

                def make_recip_step(den8, den8r):
                    def step():
                        nc.vector.reciprocal(den8r, den8)
                    return step

                def make_pair_step(p, qsl, den8r, o_raws):
                    def step():
                        rec2 = recp.tile([2, QT], BF16, tag="rec2",
                                         name="rec2")
                        nc.sync.dma_start(out=rec2,
                                          in_=den8r[2 * p:2 * p + 2, :])
                        rb_ps = prb.tile([128, QT], F32, tag="rb_ps",
                                         name="rb_ps")
                        nc.tensor.matmul(out=rb_ps, lhsT=sel_sb, rhs=rec2,
                                         start=True, stop=True)
                        for h in range(2):
                            rbh = rbp.tile([DH, QT], BF16, tag="rb",
                                           name=f"rb{h}")
                            nc.vector.tensor_copy(
                                rbh, rb_ps[h * DH:(h + 1) * DH, :])
                            nc.vector.tensor_mul(
                                oT[p][h * DH:(h + 1) * DH, qsl],
                                o_raws[2 * p + h], rbh)
                    return step

                def pump(n):
                    for _ in range(n):
                        if pend_norm:
                            pend_norm.pop(0)()
                        else:
                            d_pump(1)

                for t_i in range(NQT):
                    qsl = slice(t_i * QT, (t_i + 1) * QT)
                    nch = (t_i + 1) * NQT
                    if t_i > 0:
                        d_pending.extend(
                            ((t_i - 1) * NQT + u, nb)
                            for u in range(NQT) for nb in range(2))
                    den8 = denp.tile([2 * NPAIRS, QT], F32, tag="den8",
                                     bufs=2)
                    o_raws = []
                    for p in range(NPAIRS):
                        o_ps = [pso.tile([DH + 1, QT], F32, tag="o_ps",
                                         name=f"o_ps{h}") for h in range(2)]
                        prev = None
                        for c in range(nch):
                            cur = scores_chunk(p, t_i, c, nch)
                            if prev is not None:
                                pv_chunk(p, c - 1, nch, o_ps, *prev)
                            prev = cur
                            pump(1)
                        pv_chunk(p, nch - 1, nch, o_ps, *prev)

                        # quick copies free the o_ps ring: raw O rows (bf16)
                        # and the denominator row (f32, DMA'd into the
                        # per-tile batch for one shared reciprocal)
                        for h in range(2):
                            dr = denp.tile([1, QT], F32, tag="den_row")
                            nc.vector.tensor_copy(dr, o_ps[h][DH:DH + 1, :])
                            nc.sync.dma_start(
                                out=den8[2 * p + h:2 * p + h + 1, :], in_=dr)
                            orh = orp.tile([DH, QT], BF16, tag="o_raw",
                                           name=f"o_raw{h}")
                            nc.vector.tensor_copy(orh, o_ps[h][0:DH, :])
                            o_raws.append(orh)

                    den8r = denp.tile([2 * NPAIRS, QT], BF16, tag="den8r",
                                      bufs=2)
                    pend_norm.append(make_recip_step(den8, den8r))
                    for p in range(NPAIRS):
                        pend_norm.append(
                            make_pair_step(p, qsl, den8r, o_raws))
                # flush: last tile's norms, then its out-proj units
                pump(len(pend_norm))
                d_pending.extend(
                    ((NQT - 1) * NQT + u, nb)
                    for u in range(NQT) for nb in range(2))
                d_pump(len(d_pending) * NPAIRS)

    return nc


def make_masks():
    import ml_dtypes

    j = np.arange(KC)[:, None]
    i = np.arange(QT)[None, :]
    return np.stack(
        [(i >= j + 128 * m) for m in range(4)]).astype(ml_dtypes.bfloat16)


def make_in_maps(x, ln_gamma, ln_beta, w_qkv, w_out):
    import ml_dtypes

    bf16 = ml_dtypes.bfloat16
    x = np.asarray(x, np.float32)
    g_ = np.asarray(ln_gamma, np.float32)
    b_ = np.asarray(ln_beta, np.float32)
    w_qkv = np.asarray(w_qkv, np.float32)
    w_out = np.asarray(w_out, np.float32)
    masks = make_masks()
    in_maps = []
    for c in range(8):
        b = c // 2
        g = c % 2
        cs = slice(g * INNER_PC, (g + 1) * INNER_PC)
        Wraw = np.concatenate(
            [w_qkv[:, 0 * DIM:1 * DIM][:, cs],
             w_qkv[:, 1 * DIM:2 * DIM][:, cs],
             w_qkv[:, 2 * DIM:3 * DIM][:, cs]], axis=1)
        Wp = Wraw * g_[:, None]
        bqkv = b_ @ Wraw  # [3*INNER_PC]
        betaqk = np.empty((128, 2 * NPAIRS), np.float32)
        for p in range(NPAIRS):
            betaqk[:, 2 * p] = bqkv[p * 128:(p + 1) * 128]
            betaqk[:, 2 * p + 1] = bqkv[INNER_PC + p * 128:
                                        INNER_PC + (p + 1) * 128]
        betav = np.ascontiguousarray(
            bqkv[2 * INNER_PC:3 * INNER_PC][None, :]).astype(np.float32)
        selm = np.zeros((2, 128), np.float32)
        selm[0, 0:64] = 1.0
        selm[1, 64:128] = 1.0
        in_maps.append({
            "xT": np.ascontiguousarray(x[b].T).astype(bf16),
            "w": Wp.astype(bf16),
            "wo": np.ascontiguousarray(w_out[cs, :]).astype(bf16),
            "masks": masks,
            "betaqk": betaqk,
            "betav": betav,
            "selm": selm.astype(bf16),
        })
    return in_maps


_PROG = None


def kernel(x, ln_gamma, ln_beta, w_qkv, w_out):
    global _PROG
    from concourse.bass_utils import run_bass_kernel_spmd

    if _PROG is None:
        _PROG = build_program()
    in_maps = make_in_maps(x, ln_gamma, ln_beta, w_qkv, w_out)
    res = run_bass_kernel_spmd(_PROG, in_maps, list(range(8)))
    parts = [res.results[c]["out"] for c in range(8)]
    out = np.empty((B, TOK, DIM), np.float32)
    for b in range(B):
        out[b] = parts[2 * b] + parts[2 * b + 1]
    return out


# revision 34
# speedup vs baseline: 1.8462x; 1.0222x over previous
"""Causal attention block (LN -> QKV -> causal MHA -> out-proj) on 8 trn2
NeuronCores via Bass/Tile.

Sharding: core c handles batch b=c//2 and head-group g=c%2 (8 of 16 heads).
Data parallel over batch, tensor parallel over heads; the out-proj partial
sums (2 per batch) are reduced on the host during the gather, so the device
program needs no collectives and is pure SPMD.

Per-core pipeline (all matmul operands bf16, fp32 PSUM accumulation):
  A) x^T arrives host-transposed d-major in bf16. LN stats via ones-column
     matmuls into [1, 2048] PSUM rows; the rows are PE-broadcast (with the
     1/D mean division folded into the broadcast operand) to [128, 512]
     tiles and the whole mu/var/rstd postprocess runs on those wide tiles.
     xn = x*rstd - mu*rstd is materialized in place (2 DVE ops per tile,
     earliest token slice first so phase B can start), which removes every
     seed matmul from the projections.
  B) V (token-major, with a 65th ones column for the softmax denominator)
     and Q^T/K^T for all 4 head pairs. QK PSUM->SBUF copies run on the ACT
     engine as Identity-with-bias (applies the LN beta term for free).
  C) Attention per query tile x head pair, software-pipelined so the PE
     never waits on the ACT exp: scores of chunk c+1 are emitted before
     P@V of chunk c. Diagonal chunks compute/exp/mask only the causally
     live [128m:512] sub-rectangle. The softmax denominator rides as a
     65th ones-column of V; normalization is decoupled from the PSUM ring
     by quick copies, fast approximate reciprocals, and one K=2 select
     matmul per pair that broadcasts both heads' reciprocal rows.
  D) The out projection is interleaved one matmul at a time into the
     attention chunk stream of the following query tile (a small state
     machine pumps it), filling PE slack while ACT paces the exps.
"""

import numpy as np

import concourse.bass as bass
import concourse.mybir as mybir
import concourse.tile as tile_mod

# ----------------------------------------------------------------------------
# Workaround for this walrus build rejecting instructions that carry more than
# MAX_WAITS semaphore waits ("Too many sync wait commands" in CoreV3GenImpl
# setupSyncWait — hit on Drain and Matmult/S3_LW encodings). Split excess
# waits onto single-wait NOP carrier instructions emitted just before the
# original instruction on the same engine: program order on the sequencer
# makes this semantically identical (waits are AND conditions).
# ----------------------------------------------------------------------------
_MAX_WAITS = 1
_orig_add_instruction = tile_mod.TileContext._add_instruction
_carrier_id = [0]


def _split_waits_add_instruction(self, inst):
    si = inst.sync_info
    if (
        si is not None
        and si.on_wait
        and len(si.on_wait) > _MAX_WAITS
        and inst.engine != mybir.EngineType.Unassigned
    ):
        waits = list(si.on_wait)
        keep = waits[-_MAX_WAITS:]
        for w in waits[:-_MAX_WAITS]:
            _carrier_id[0] += 1
            nop = mybir.InstNoOp(name=f"I-waitc-{_carrier_id[0]}")
            nop.engine = inst.engine
            nop.sync_info = mybir.SyncInfo(on_wait=[w], on_update=[])
            _orig_add_instruction(self, nop)
        inst.sync_info = mybir.SyncInfo(
            on_wait=keep,
            on_update=list(si.on_update) if si.on_update else [],
        )
    _orig_add_instruction(self, inst)


tile_mod.TileContext._add_instruction = _split_waits_add_instruction

from concourse.vector_clock import ScopedClock


def _patched_drain_and_barrier(self, tick_clock, wait_clock):
    # Same wait-splitting for the TileContext exit drain, which is emitted
    # after lowering (outside _add_instruction).
    nc = self.nc
    carrier = nc.sync.nop(nofuse=True)
    wait_clock.add_sem_waits(carrier.ins, ScopedClock({None: tick_clock.global_clock}))
    si = carrier.ins.sync_info
    waits = list(si.on_wait) if si is not None and si.on_wait else []
    if len(waits) > _MAX_WAITS:
        carrier.ins.sync_info = mybir.SyncInfo(
            on_wait=waits[:_MAX_WAITS],
            on_update=list(si.on_update) if si.on_update else [],
        )
        rest = waits[_MAX_WAITS:]
        while rest:
            extra = nc.sync.nop(nofuse=True)
            extra.ins.sync_info = mybir.SyncInfo(
                on_wait=rest[:_MAX_WAITS], on_update=[])
            rest = rest[_MAX_WAITS:]

    nc.sync.drain()
    nc.all_engine_barrier()
    assert self.sems is not None
    popped = nc._tile_sem_poison_stack.pop()
    assert popped is self._sem_poison
    nc.clear_and_free_semaphores(list(self.sems.allocated().values()))
    nc.all_engine_barrier()


tile_mod.TileContext._drain_and_barrier = _patched_drain_and_barrier

# ----------------------------------------------------------------------------

F32 = mybir.dt.float32
F32R = mybir.dt.float32r
BF16 = mybir.dt.bfloat16
ALU = mybir.AluOpType
ACT_EXP = mybir.ActivationFunctionType.Exp
ACT_SQRT = mybir.ActivationFunctionType.Sqrt
ACT_IDENT = mybir.ActivationFunctionType.Identity
U32 = mybir.dt.uint32
ONE_BITS = int(np.float32(1.0).view(np.uint32))

B = 4
TOK = 2048
DIM = 1024
HEADS = 16
DH = 64
HEADS_PC = 8          # heads per core
INNER_PC = HEADS_PC * DH  # 512
NPAIRS = HEADS_PC // 2
QT = 512              # query tile (matmul moving dim)
KC = 128              # key-token chunk (contraction tile)
NKD = DIM // 128      # d-contraction chunks
NQT = TOK // QT       # query tiles
NTT = TOK // KC       # 128-token tiles
EPS = 1e-5
SCALE = DH ** -0.5


def build_program():
    nc = bass.Bass()
    xT = nc.declare_dram_parameter("xT", [DIM, TOK], BF16, isOutput=False)
    w = nc.declare_dram_parameter("w", [DIM, 3 * INNER_PC], BF16, isOutput=False)
    wo = nc.declare_dram_parameter("wo", [INNER_PC, DIM], BF16, isOutput=False)
    masks = nc.declare_dram_parameter("masks", [4, KC, QT], BF16, isOutput=False)
    betaqk = nc.declare_dram_parameter("betaqk", [128, 2 * NPAIRS], F32,
                                       isOutput=False)
    betav = nc.declare_dram_parameter("betav", [1, INNER_PC], F32R,
                                      isOutput=False)
    selm = nc.declare_dram_parameter("selm", [2, 128], BF16, isOutput=False)
    out = nc.declare_dram_parameter("out", [TOK, DIM], F32, isOutput=True)

    import contextlib
    with tile_mod.TileContext(nc) as tc, nc.allow_low_precision(
            "bf16 operand tiles; all matmul accumulation stays fp32 PSUM"):
        with contextlib.ExitStack() as _st:
            const = _st.enter_context(tc.tile_pool(name="const", bufs=1))
            wop = _st.enter_context(tc.tile_pool(name="wo_sb", bufs=NPAIRS))
            vpool = _st.enter_context(tc.tile_pool(name="vpool", bufs=NTT))
            qkp = _st.enter_context(tc.tile_pool(name="qkT", bufs=NPAIRS))
            otp = _st.enter_context(tc.tile_pool(name="oT", bufs=NPAIRS))
            bcp = _st.enter_context(tc.tile_pool(name="bc", bufs=NQT))
            # ---------------- constants + full prefetch ----------------
            ones_col = const.tile([128, 1], BF16, tag="ones_col")
            nc.vector.memset(ones_col, 1.0)
            ones_row = const.tile([1, 128], F32R, tag="ones_row")
            nc.vector.memset(ones_row.bitcast(U32), ONE_BITS)
            inv_bf = const.tile([1, 128], BF16, tag="inv_bf")
            nc.vector.memset(inv_bf, 1.0 / DIM)
            eps128 = const.tile([128, 1], F32, tag="eps")
            nc.vector.memset(eps128, EPS)
            bqk_sb = const.tile([128, 2 * NPAIRS], F32, tag="bqk")
            nc.sync.dma_start(out=bqk_sb, in_=betaqk[:, :])
            # head-select matrix for the two-head reciprocal broadcast
            sel_sb = const.tile([2, 128], BF16, tag="sel")
            nc.sync.dma_start(out=sel_sb, in_=selm[:, :])
            bv_row = const.tile([1, INNER_PC], F32R, tag="bv_row")
            nc.sync.dma_start(out=bv_row, in_=betav[:, :])
            mask_sb = const.tile([KC, 4, QT], BF16, tag="mask")
            for m in range(4):
                nc.sync.dma_start(out=mask_sb[:, m, :], in_=masks[m, :, :])

            _stAB = contextlib.ExitStack()
            xtp = _stAB.enter_context(tc.tile_pool(name="xt", bufs=NKD))
            wsp = _stAB.enter_context(tc.tile_pool(name="wsb", bufs=NKD))
            xt = []
            for kc in range(NKD):
                t = xtp.tile([128, TOK], BF16, tag="xt")
                for nt in range(NQT):
                    sl = slice(nt * QT, (nt + 1) * QT)
                    nc.sync.dma_start(out=t[:, sl],
                                      in_=xT[kc * 128:(kc + 1) * 128, sl])
                xt.append(t)
            wsb = []
            for kc in range(NKD):
                t = wsp.tile([128, 3 * INNER_PC], BF16, tag="wsb")
                nc.sync.dma_start(out=t, in_=w[kc * 128:(kc + 1) * 128, :])
                wsb.append(t)
            wos = []
            for p in range(NPAIRS):
                t = wop.tile([128, DIM], BF16, tag="wo_sb")
                nc.sync.dma_start(out=t, in_=wo[p * 128:(p + 1) * 128, :])
                wos.append(t)

            # persistent per-pair tiles
            qT = [qkp.tile([128, TOK], BF16, tag="qT", name=f"qT{p}")
                  for p in range(NPAIRS)]
            kT = [qkp.tile([128, TOK], BF16, tag="kT", name=f"kT{p}")
                  for p in range(NPAIRS)]
            oT = [otp.tile([128, TOK], BF16, tag="oT", name=f"oT{p}")
                  for p in range(NPAIRS)]
            bv_bc = const.tile([128, INNER_PC], F32, tag="bv_bc")

            # ---------------- phase A: LN stats + xn ----------------
            with (
                tc.tile_pool(name="sqp", bufs=3) as sqp,
            ):
                sum_r = sqp.tile([1, TOK], F32, tag="sum_r", bufs=1)
                sq_r = sqp.tile([1, TOK], F32, tag="sq_r", bufs=1)
                with tc.tile_pool(name="ps_stats", bufs=1,
                                  space="PSUM") as pstat:
                    sum_ps = pstat.tile([1, TOK], F32, tag="sum")
                    sq_ps = pstat.tile([1, TOK], F32, tag="sq")
                    for kc in range(NKD):
                        for nt in range(NQT):
                            sl = slice(nt * QT, (nt + 1) * QT)
                            sq_t = sqp.tile([128, QT], BF16, tag="sq_t")
                            nc.vector.tensor_mul(sq_t, xt[kc][:, sl],
                                                 xt[kc][:, sl])
                            nc.tensor.matmul(
                                out=sum_ps[0:1, sl], lhsT=ones_col,
                                rhs=xt[kc][:, sl],
                                start=(kc == 0), stop=(kc == NKD - 1))
                            nc.tensor.matmul(
                                out=sq_ps[0:1, sl], lhsT=ones_col,
                                rhs=sq_t,
                                start=(kc == 0), stop=(kc == NKD - 1))
                    for nt in range(NQT):
                        sl = slice(nt * QT, (nt + 1) * QT)
                        nc.vector.tensor_copy(sum_r[0:1, sl], sum_ps[0:1, sl])
                        nc.scalar.copy(sq_r[0:1, sl], sq_ps[0:1, sl])

                # compact [128, 16] stats chain (partition-parallel, so the
                # expensive reciprocal runs on 16 elements per lane), then
                # DMA-reshape back to f32r rows for the PE broadcasts
                bc_rstd, bc_nmr = [], []
                with (
                    tc.tile_pool(name="ps_bc", bufs=4, space="PSUM") as pbc,
                    tc.tile_pool(name="stw", bufs=1) as stw,
                ):
                    CW = TOK // 128  # 16
                    sum_c = stw.tile([128, CW], F32, tag="sum_c")
                    nc.sync.dma_start(out=sum_c, in_=sum_r[0:1, :])
                    sq_c = stw.tile([128, CW], F32, tag="sq_c")
                    nc.sync.dma_start(out=sq_c, in_=sq_r[0:1, :])
                    mu_c = stw.tile([128, CW], F32, tag="mu_c")
                    nc.vector.tensor_scalar_mul(mu_c, sum_c, 1.0 / DIM)
                    var_c = stw.tile([128, CW], F32, tag="var_c")
                    nc.scalar.square(var_c, mu_c)
                    ex_c = stw.tile([128, CW], F32, tag="ex_c")
                    nc.vector.tensor_scalar_mul(ex_c, sq_c, 1.0 / DIM)
                    nc.vector.tensor_sub(var_c, ex_c, var_c)
                    nc.scalar.activation(out=var_c, in_=var_c, func=ACT_SQRT,
                                         bias=eps128, scale=1.0)  # std
                    rstd_c = stw.tile([128, CW], F32R, tag="rstd_c")
                    nc.vector.reciprocal(rstd_c, var_c)
                    nmr_c = stw.tile([128, CW], F32R, tag="nmr_c")
                    nc.vector.tensor_mul(nmr_c, mu_c, rstd_c)  # mu*rstd
                    rstd_row = stw.tile([1, TOK], F32R, tag="rstd_row")
                    nc.sync.dma_start(out=rstd_row[0:1, :], in_=rstd_c)
                    nmr_row = stw.tile([1, TOK], F32R, tag="nmr_row")
                    nc.sync.dma_start(out=nmr_row[0:1, :], in_=nmr_c)
                    for nt in range(NQT):
                        sl = slice(nt * QT, (nt + 1) * QT)
                        for src_row, dstl, tg, eng in (
                            (rstd_row, bc_rstd, "bc_rstd", "v"),
                            (nmr_row, bc_nmr, "bc_nmr", "s"),
                        ):
                            ps = pbc.tile([128, QT], F32, tag="bc_ps",
                                          name="ps")
                            nc.tensor.matmul(out=ps, lhsT=ones_row,
                                             rhs=src_row[0:1, sl],
                                             start=True, stop=True)
                            t = bcp.tile([128, QT], BF16, tag=tg, name="t")
                            if eng == "v":
                                nc.vector.tensor_copy(t, ps)
                            else:
                                nc.scalar.copy(t, ps)
                            dstl.append(t)
                    bv_ps = pbc.tile([128, INNER_PC], F32, tag="bv_ps",
                                     bufs=1)
                    nc.tensor.matmul(out=bv_ps, lhsT=ones_row,
                                     rhs=bv_row, start=True, stop=True)
                    nc.vector.tensor_copy(bv_bc, bv_ps)

                # xn = x*rstd - mu*rstd, in place; token-slice-major order so
                # phase B can start on slice 0 early
                for nt in range(NQT):
                    sl = slice(nt * QT, (nt + 1) * QT)
                    for kc in range(NKD):
                        nc.vector.tensor_mul(xt[kc][:, sl], xt[kc][:, sl],
                                             bc_rstd[nt])
                        nc.vector.tensor_sub(xt[kc][:, sl], xt[kc][:, sl],
                                             bc_nmr[nt])

            # ---------------- phase B: V then Q^T/K^T projections ----------
            with (
                tc.tile_pool(name="ps_v", bufs=2, space="PSUM") as psv,
                tc.tile_pool(name="ps_qk", bufs=3, space="PSUM") as psqk,
            ):
                v_sb = [None] * NTT
                for nt in range(NQT):
                    sl = slice(nt * QT, (nt + 1) * QT)
                    for tt in range(4 * nt, 4 * nt + 4):
                        tsl = slice(tt * KC, (tt + 1) * KC)
                        v_ps = psv.tile([128, INNER_PC], F32, tag="v_ps")
                        for kc in range(NKD):
                            nc.tensor.matmul(
                                out=v_ps, lhsT=xt[kc][:, tsl],
                                rhs=wsb[kc][:, 2 * INNER_PC:3 * INNER_PC],
                                start=(kc == 0), stop=(kc == NKD - 1))
                        vt = vpool.tile([128, HEADS_PC * (DH + 1)], BF16,
                                        tag="v_sb")
                        v3 = vt.rearrange("p (h w) -> p h w", w=DH + 1)
                        nc.vector.tensor_add(
                            v3[:, :, 0:DH],
                            v_ps.rearrange("p (h w) -> p h w", w=DH),
                            bv_bc.rearrange("p (h w) -> p h w", w=DH))
                        nc.vector.memset(v3[:, :, DH:DH + 1], 1.0)
                        v_sb[tt] = vt
                    for p in range(NPAIRS):
                        for di, (dst, cofs) in enumerate(
                            ((qT[p], p * 128), (kT[p], INNER_PC + p * 128)),
                        ):
                            ps = psqk.tile([128, QT], F32, tag="qk_ps")
                            for kc in range(NKD):
                                nc.tensor.matmul(
                                    out=ps,
                                    lhsT=wsb[kc][:, cofs:cofs + 128],
                                    rhs=xt[kc][:, sl],
                                    start=(kc == 0), stop=(kc == NKD - 1))
                            nc.scalar.activation(
                                out=dst[:, sl], in_=ps, func=ACT_IDENT,
                                bias=bqk_sb[:, 2 * p + di:2 * p + di + 1],
                                scale=1.0)

            _stAB.close()

            # -------- phases C+D: attention with interleaved out-proj -------
            with contextlib.ExitStack() as _stC:
                pss = _stC.enter_context(
                    tc.tile_pool(name="ps_s", bufs=2, space="PSUM"))
                pso = _stC.enter_context(
                    tc.tile_pool(name="ps_o", bufs=2, space="PSUM"))
                prb = _stC.enter_context(
                    tc.tile_pool(name="ps_rb", bufs=1, space="PSUM"))
                pmisc = _stC.enter_context(
                    tc.tile_pool(name="ps_misc", bufs=1, space="PSUM"))
                ppool = _stC.enter_context(tc.tile_pool(name="psb", bufs=5))
                orp = _stC.enter_context(tc.tile_pool(name="orp", bufs=16))
                rbp = _stC.enter_context(tc.tile_pool(name="rbp", bufs=2))
                recp = _stC.enter_context(tc.tile_pool(name="rec", bufs=4))
                denp = _stC.enter_context(tc.tile_pool(name="den", bufs=8))
                outp = _stC.enter_context(tc.tile_pool(name="out_sb", bufs=3))
                # --- out-proj state machine: one matmul per pump step ---
                d_pending = []   # queue of token-128-tile x dim-half units
                d_state = {"ps": None, "step": 0, "i": 0, "rr": None}

                def d_pump(nsteps, flush=False):
                    for _ in range(nsteps):
                        if not d_pending:
                            return
                        tt, nb = d_pending[0]
                        tsl = slice(tt * KC, (tt + 1) * KC)
                        nsl = slice(nb * QT, (nb + 1) * QT)
                        if d_state["ps"] is None:
                            if flush:
                                pool, tg = d_state["rr"][d_state["i"] % 3]
                                d_state["i"] += 1
                            else:
                                pool, tg = pmisc, "d_ps"
                            d_state["ps"] = pool.tile([128, QT], F32,
                                                      tag=tg,
                                                      name="d_ps")
                            d_state["step"] = 0
                        p_i = d_state["step"]
                        nc.tensor.matmul(
                            out=d_state["ps"], lhsT=oT[p_i][:, tsl],
                            rhs=wos[p_i][:, nsl],
                            start=(p_i == 0), stop=(p_i == NPAIRS - 1))
                        d_state["step"] += 1
                        if d_state["step"] == NPAIRS:
                            ob = outp.tile([128, QT], F32, tag="out_sb")
                            nc.vector.tensor_copy(ob, d_state["ps"])
                            nc.sync.dma_start(out=out[tsl, nsl], in_=ob)
                            d_state["ps"] = None
                            d_pending.pop(0)

                def scores_chunk(p, t_i, c, nch):
                    """Emit scores matmuls + exp + mask for chunk c; returns
                    the p_sb tile and the live q-range start (for PV)."""
                    qsl = slice(t_i * QT, (t_i + 1) * QT)
                    csl = slice(c * KC, (c + 1) * KC)
                    m = c - (nch - 4)
                    lo = 128 * m if m >= 1 else 0   # live q-range start
                    s_ps = pss.tile([128, 2 * QT], F32, tag="s_ps")
                    qlo = slice(t_i * QT + lo, (t_i + 1) * QT)
                    nc.tensor.matmul(
                        out=s_ps[:, lo:QT],
                        lhsT=kT[p][0:DH, csl], rhs=qT[p][0:DH, qlo],
                        start=True, stop=True)
                    nc.tensor.matmul(
                        out=s_ps[:, QT + lo:2 * QT],
                        lhsT=kT[p][DH:128, csl], rhs=qT[p][DH:128, qlo],
                        start=True, stop=True)
                    p_sb = ppool.tile([128, 2 * QT], BF16, tag="p_sb")
                    s3 = s_ps.rearrange("p (h q) -> p h q", q=QT)
                    p3 = p_sb.rearrange("p (h q) -> p h q", q=QT)
                    nc.scalar.activation(out=p3[:, :, lo:QT],
                                         in_=s3[:, :, lo:QT],
                                         func=ACT_EXP, scale=SCALE)
                    if m >= 0:
                        for h in range(2):
                            nc.vector.tensor_mul(
                                p_sb[:, h * QT + lo:(h + 1) * QT],
                                p_sb[:, h * QT + lo:(h + 1) * QT],
                                mask_sb[:, m, lo:QT])
                    return p_sb, lo

                def pv_chunk(p, c, nch, o_ps, p_sb, lo):
                    for h in range(2):
                        hc = (2 * p + h) * (DH + 1)
                        nc.tensor.matmul(
                            out=o_ps[h][:, lo:QT],
                            lhsT=v_sb[c][:, hc:hc + DH + 1],
                            rhs=p_sb[:, h * QT + lo:(h + 1) * QT],
                            start=(c == 0), stop=(c == nch - 1),
                            skip_group_check=True)

                # normalize steps for tile t_i are deferred and pumped
                # into tile t_i+1's chunk stream, ahead of that tile's
                # out-proj units (norms write the oT slices D reads)
                pend_norm = []

                def make_recip_step(den8, den8r):
                    def step():
                        nc.vector.reciprocal(den8r, den8)
                    return step

                def make_pair_step(p, qsl, den8r, o_raws):
                    def step():
                        rec2 = recp.tile([2, QT], BF16, tag="rec2",
                                         name="rec2")
                        nc.sync.dma_start(out=rec2,
                                          in_=den8r[2 * p:2 * p + 2, :])
                        rb_ps = prb.tile([128, QT], F32, tag="rb_ps",
                                         name="rb_ps")
                        nc.tensor.matmul(out=rb_ps, lhsT=sel_sb, rhs=rec2,
                                         start=True, stop=True)
                        for h in range(2):
                            rbh = rbp.tile([DH, QT], BF16, tag="rb",
                                           name=f"rb{h}")
                            nc.vector.tensor_copy(
                                rbh, rb_ps[h * DH:(h + 1) * DH, :])
                            nc.vector.tensor_mul(
                                oT[p][h * DH:(h + 1) * DH, qsl],
                                o_raws[2 * p + h], rbh)
                    return step

                def pump(n):
                    for _ in range(n):
                        if pend_norm:
                            pend_norm.pop(0)()
                        else:
                            d_pump(1)

                for t_i in range(NQT):
                    qsl = slice(t_i * QT, (t_i + 1) * QT)
                    nch = (t_i + 1) * NQT
                    if t_i > 0:
                        d_pending.extend(
                            ((t_i - 1) * NQT + u, nb)
                            for u in range(NQT) for nb in range(2))
                    den8 = denp.tile([2 * NPAIRS, QT], F32, tag="den8",
                                     bufs=2)
                    o_raws = []
                    for p in range(NPAIRS):
                        o_ps = [pso.tile([DH + 1, QT], F32, tag="o_ps",
                                         name=f"o_ps{h}") for h in range(2)]
                        pend = []
                        for c in range(nch):
                            pend.append(scores_chunk(p, t_i, c, nch))
                            if len(pend) > 2:
                                pv_chunk(p, c - 2, nch, o_ps,
                                         *pend.pop(0))
                            pump(1)
                        for i, pr in enumerate(pend):
                            pv_chunk(p, nch - len(pend) + i, nch, o_ps, *pr)

                        # quick copies free the o_ps ring: raw O rows (bf16)
                        # and the denominator row (f32, DMA'd into the
                        # per-tile batch for one shared reciprocal)
                        for h in range(2):
                            dr = denp.tile([1, QT], F32, tag="den_row")
                            nc.vector.tensor_copy(dr, o_ps[h][DH:DH + 1, :])
                            nc.sync.dma_start(
                                out=den8[2 * p + h:2 * p + h + 1, :], in_=dr)
                            orh = orp.tile([DH, QT], BF16, tag="o_raw",
                                           name=f"o_raw{h}")
                            nc.vector.tensor_copy(orh, o_ps[h][0:DH, :])
                            o_raws.append(orh)

                    den8r = denp.tile([2 * NPAIRS, QT], BF16, tag="den8r",
                                      bufs=2)
                    pend_norm.append(make_recip_step(den8, den8r))
                    for p in range(NPAIRS):
                        pend_norm.append(
                            make_pair_step(p, qsl, den8r, o_raws))
                # flush: last tile's norms, then its out-proj units
                pump(len(pend_norm))
                d_pending.extend(
                    ((NQT - 1) * NQT + u, nb)
                    for u in range(NQT) for nb in range(2))
                d_state["rr"] = [(pmisc, "d_ps"), (pso, "o_ps"),
                                 (prb, "rb_ps")]
                d_pump(len(d_pending) * NPAIRS, flush=True)

    return nc


def make_masks():
    import ml_dtypes

    j = np.arange(KC)[:, None]
    i = np.arange(QT)[None, :]
    return np.stack(
        [(i >= j + 128 * m) for m in range(4)]).astype(ml_dtypes.bfloat16)


def make_in_maps(x, ln_gamma, ln_beta, w_qkv, w_out):
    import ml_dtypes

    bf16 = ml_dtypes.bfloat16
    x = np.asarray(x, np.float32)
    g_ = np.asarray(ln_gamma, np.float32)
    b_ = np.asarray(ln_beta, np.float32)
    w_qkv = np.asarray(w_qkv, np.float32)
    w_out = np.asarray(w_out, np.float32)
    masks = make_masks()
    in_maps = []
    for c in range(8):
        b = c // 2
        g = c % 2
        cs = slice(g * INNER_PC, (g + 1) * INNER_PC)
        Wraw = np.concatenate(
            [w_qkv[:, 0 * DIM:1 * DIM][:, cs],
             w_qkv[:, 1 * DIM:2 * DIM][:, cs],
             w_qkv[:, 2 * DIM:3 * DIM][:, cs]], axis=1)
        Wp = Wraw * g_[:, None]
        bqkv = b_ @ Wraw  # [3*INNER_PC]
        betaqk = np.empty((128, 2 * NPAIRS), np.float32)
        for p in range(NPAIRS):
            betaqk[:, 2 * p] = bqkv[p * 128:(p + 1) * 128]
            betaqk[:, 2 * p + 1] = bqkv[INNER_PC + p * 128:
                                        INNER_PC + (p + 1) * 128]
        betav = np.ascontiguousarray(
            bqkv[2 * INNER_PC:3 * INNER_PC][None, :]).astype(np.float32)
        selm = np.zeros((2, 128), np.float32)
        selm[0, 0:64] = 1.0
        selm[1, 64:128] = 1.0
        in_maps.append({
            "xT": np.ascontiguousarray(x[b].T).astype(bf16),
            "w": Wp.astype(bf16),
            "wo": np.ascontiguousarray(w_out[cs, :]).astype(bf16),
            "masks": masks,
            "betaqk": betaqk,
            "betav": betav,
            "selm": selm.astype(bf16),
        })
    return in_maps


_PROG = None


def kernel(x, ln_gamma, ln_beta, w_qkv, w_out):
    global _PROG
    from concourse.bass_utils import run_bass_kernel_spmd

    if _PROG is None:
        _PROG = build_program()
    in_maps = make_in_maps(x, ln_gamma, ln_beta, w_qkv, w_out)
    res = run_bass_kernel_spmd(_PROG, in_maps, list(range(8)))
    parts = [res.results[c]["out"] for c in range(8)]
    out = np.empty((B, TOK, DIM), np.float32)
    for b in range(B):
        out[b] = parts[2 * b] + parts[2 * b + 1]
    return out
